# revision 36
# baseline (speedup 1.0000x reference)
"""AttentionBlock Trainium2 Bass kernel.

Full inputs -> shard batch over 8 NeuronCores (4 samples each) -> full output.

Per-sample on-core pipeline:
  x [C=128p, HW=1024f] -> groupnorm (bn_stats + PE group-reduce + affine)
  -> Q,K (per-head channel layout, scale^2 folded into Wq), V computed
     pre-transposed (Vt[s, (h,d)])
  -> S'[s,t] = K^T Q per head via K=32 matmuls row-tiled into PE quadrants;
     exp on ScalarE (no max subtraction: |S| < 1.3)
  -> hout via M=32 col-tiled matmuls (4 heads concurrent, standard channel
     order) + Z via M=1 ones-matmuls, accumulating over s-chunks in PSUM
  -> normalize with 1/Z (packed DVE reciprocal + PE broadcast)
  -> proj (+bias via K=1 ones matmul) + residual add -> y

float32r (single-pass fp32 matmul, bf16-rounded multiply) is used for the
qkv/attention/proj matmuls; groupnorm statistics stay exact fp32.
"""

import numpy as np
from contextlib import ExitStack

B, C, HW = 32, 128, 1024
NH, DH = 4, 32
GROUPS = 32
EPS = 1e-5
NCORES = 8
BPC = B // NCORES  # samples per core

_CACHE = {}
TRACE = False
LAST_RESULT = None
# 0 = all fp32; 1 = f32r attention+proj; 2 = + qkv/groupnorm-apply
RLEVEL = 2


def _patch_tile_waits(tile, mybir):
    """This walrus build encodes only one sync-wait slot per instruction;
    Tile can attach several. Split extra waits onto NoOps committed
    immediately before the instruction on the same engine queue
    (in-order => identical semantics)."""
    if getattr(tile.TileContext, "_mm_wait_patched", False):
        return
    orig = tile.TileContext._commit_instruction

    def patched(self, inst, lazy_reg_writes=True):
        si = getattr(inst, "sync_info", None)
        if (not isinstance(inst, mybir.InstNoOp) and si is not None
                and si.on_wait and len(si.on_wait) > 1):
            waits = list(si.on_wait)
            for w in waits[:-1]:
                nop = mybir.InstNoOp(
                    name=self.nc.get_next_instruction_name(),
                    engine=inst.engine,
                    bass_nofuse=True,
                    sync_info=mybir.SyncInfo(on_wait=[w], on_update=[]),
                )
                orig(self, nop, lazy_reg_writes=False)
            inst.sync_info = mybir.SyncInfo(
                on_wait=[waits[-1]], on_update=list(si.on_update))
        return orig(self, inst, lazy_reg_writes)

    tile.TileContext._commit_instruction = patched

    def patched_drain(self, tick_clock, wait_clock):
        # Collect end-of-kernel waits, then hand them out one per SP nop
        # (the drain keeps none); nops precede the teardown barrier on the
        # same queue, so semantics are preserved.
        self.nc.sync.drain()
        sink = self.nc.sync.nop(nofuse=True)
        wait_clock.add_sem_waits(
            sink.ins, tile.ScopedClock({None: tick_clock.global_clock}))
        si = sink.ins.sync_info
        waits = list(si.on_wait) if si and si.on_wait else []
        if len(waits) > 1:
            sink.ins.sync_info = mybir.SyncInfo(
                on_wait=[waits[0]], on_update=list(si.on_update))
            for w in waits[1:]:
                extra = self.nc.sync.nop(nofuse=True)
                extra.ins.sync_info = mybir.SyncInfo(on_wait=[w], on_update=[])

        self.nc.all_engine_barrier()
        assert self.sems is not None
        popped = self.nc._tile_sem_poison_stack.pop()
        assert popped is self._sem_poison
        self.nc.clear_and_free_semaphores(list(self.sems.allocated().values()))
        self.nc.all_engine_barrier()

    tile.TileContext._drain_and_barrier = patched_drain
    tile.TileContext._mm_wait_patched = True


def _build_nc():
    import concourse.bass as bass
    import concourse.tile as tile
    from concourse import mybir

    _patch_tile_waits(tile, mybir)

    f32 = mybir.dt.float32
    d1 = mybir.dt.float32r if RLEVEL >= 1 else f32
    d2 = mybir.dt.float32r if RLEVEL >= 2 else f32
    nc = bass.Bass()

    x_d = nc.dram_tensor("x", [BPC, C, HW], f32, kind="ExternalInput")
    wq_d = nc.dram_tensor("wqT", [C, C], d2, kind="ExternalInput")
    wk_d = nc.dram_tensor("wkT", [C, C], d2, kind="ExternalInput")
    wv_d = nc.dram_tensor("wvT", [C, C], d2, kind="ExternalInput")
    pjT_d = nc.dram_tensor("pjT", [C, C], d1, kind="ExternalInput")
    pjb_d = nc.dram_tensor("pjb", [1, C], f32, kind="ExternalInput")
    nw_d = nc.dram_tensor("nw", [C, 1], f32, kind="ExternalInput")
    nb_d = nc.dram_tensor("nb", [C, 1], f32, kind="ExternalInput")
    g1_d = nc.dram_tensor("g1", [C, GROUPS], f32, kind="ExternalInput")
    g2_d = nc.dram_tensor("g2", [GROUPS, C], f32, kind="ExternalInput")
    ebc_d = nc.dram_tensor("ebc", [NH, C], f32, kind="ExternalInput")
    y_d = nc.dram_tensor("y", [BPC, C, HW], f32, kind="ExternalOutput")

    with tile.TileContext(nc) as tc:
        with ExitStack() as ctx:
            _body(ctx, tc, mybir, bass, d1, d2,
                  x_d, wq_d, wk_d, wv_d, pjT_d, pjb_d, nw_d, nb_d,
                  g1_d, g2_d, ebc_d, y_d)
    return nc


def _body(ctx, tc, mybir, bass, d1, d2,
          x_d, wq_d, wk_d, wv_d, pjT_d, pjb_d, nw_d, nb_d, g1_d, g2_d,
          ebc_d, y_d):
    nc = tc.nc
    f32 = mybir.dt.float32
    AF = mybir.ActivationFunctionType
    OP = mybir.AluOpType

    const = ctx.enter_context(tc.tile_pool(name="const", bufs=1))
    sb_x = ctx.enter_context(tc.tile_pool(name="sb_x", bufs=2))
    sb_qk = ctx.enter_context(tc.tile_pool(name="sb_qk", bufs=2))
    sb_a = ctx.enter_context(tc.tile_pool(name="sb_a", bufs=3))
    sb_sm = ctx.enter_context(tc.tile_pool(name="sb_sm", bufs=4))
    # PSUM budget: ps_s 2x[128,1024] (4 banks) + ps_h tags h,z (4 banks) = 8
    ps_s = ctx.enter_context(tc.tile_pool(name="ps_s", bufs=2, space="PSUM"))
    ps_h = ctx.enter_context(tc.tile_pool(name="ps_h", bufs=1, space="PSUM"))

    # ---- constants ----
    wq_sb = const.tile([C, C], d2, tag="wq")
    wk_sb = const.tile([C, C], d2, tag="wk")
    wv_sb = const.tile([C, C], d2, tag="wv")
    pj_sb = const.tile([C, C], d1, tag="pj")
    pjb_sb = const.tile([1, C], f32, tag="pjb")
    nw_sb = const.tile([C, 1], f32, tag="nw")
    nb_sb = const.tile([C, 1], f32, tag="nb")
    g1_sb = const.tile([C, GROUPS], f32, tag="g1")
    g2_sb = const.tile([GROUPS, C], f32, tag="g2")
    ebc_sb = const.tile([NH, C], f32, tag="ebc")
    for dst, src in ((wq_sb, wq_d), (wk_sb, wk_d), (wv_sb, wv_d),
                     (pj_sb, pjT_d), (pjb_sb, pjb_d), (nw_sb, nw_d),
                     (nb_sb, nb_d), (g1_sb, g1_d), (g2_sb, g2_d),
                     (ebc_sb, ebc_d)):
        nc.sync.dma_start(out=dst, in_=src[:])
    ones_sb = const.tile([1, HW], f32, tag="ones")
    nc.vector.memset(ones_sb, 1.0)
    eps_sb = const.tile([GROUPS, 1], f32, tag="eps")
    nc.vector.memset(eps_sb, EPS)
    ones32 = const.tile([C, DH], f32, tag="ones32")
    nc.vector.memset(ones32, 1.0)


    NHALF = HW // 2  # 512

    for b in range(BPC):
        # ---------- load ----------
        x_sb = sb_x.tile([C, HW], f32, tag="x")
        nc.sync.dma_start(out=x_sb, in_=x_d[b])

        # ---------- groupnorm ----------
        st6 = sb_sm.tile([C, 2, 6], f32, tag="st6")
        nc.vector.bn_stats(out=st6[:, 0, :], in_=x_sb[:, 0:512])
        nc.vector.bn_stats(out=st6[:, 1, :], in_=x_sb[:, 512:1024])
        mv = sb_sm.tile([C, 2], f32, tag="mv")
        nc.vector.bn_aggr(out=mv, in_=st6)
        # s2 = [mean_c, mean_c^2 + var_c]
        s2 = sb_sm.tile([C, 2], f32, tag="s2")
        nc.vector.tensor_copy(out=s2[:, 0:1], in_=mv[:, 0:1])
        nc.vector.tensor_mul(out=s2[:, 1:2], in0=mv[:, 0:1], in1=mv[:, 0:1])
        nc.vector.tensor_add(out=s2[:, 1:2], in0=s2[:, 1:2], in1=mv[:, 1:2])
        # group reduce: [32, 2] = (g1/4)^T @ s2
        gp = ps_s.tile([C, HW], f32, tag="s")
        nc.tensor.matmul(gp[0:GROUPS, 0:2], g1_sb, s2, start=True, stop=True)
        gs = sb_sm.tile([GROUPS, 2], f32, tag="gs")
        nc.vector.tensor_copy(out=gs, in_=gp[0:GROUPS, 0:2])
        # vv = [mu_g, rstd_g]; rstd = exp(-0.5*ln(var+eps)) (same ACT set)
        vv = sb_sm.tile([GROUPS, 2], f32, tag="vv")
        nc.vector.tensor_mul(out=vv[:, 0:1], in0=gs[:, 0:1], in1=gs[:, 0:1])
        nc.vector.tensor_tensor(out=vv[:, 1:2], in0=gs[:, 1:2], in1=vv[:, 0:1],
                                op=OP.subtract)
        nc.scalar.activation(out=vv[:, 1:2], in_=vv[:, 1:2], func=AF.Ln,
                             bias=eps_sb, scale=1.0)
        nc.scalar.activation(out=vv[:, 1:2], in_=vv[:, 1:2], func=AF.Exp,
                             bias=0.0, scale=-0.5)
        nc.vector.tensor_copy(out=vv[:, 0:1], in_=gs[:, 0:1])
        # broadcast to channels: bc[c, 0:2] = [mu_c, rstd_c]
        bc = ps_s.tile([C, HW], f32, tag="s")
        nc.tensor.matmul(bc[0:C, 0:2], g2_sb, vv, start=True, stop=True)
        aff = sb_sm.tile([C, 2], f32, tag="aff")
        nc.vector.tensor_mul(out=aff[:, 0:1], in0=nw_sb, in1=bc[:, 1:2])
        nc.vector.tensor_mul(out=aff[:, 1:2], in0=bc[:, 0:1], in1=aff[:, 0:1])
        nc.vector.tensor_tensor(out=aff[:, 1:2], in0=nb_sb, in1=aff[:, 1:2],
                                op=OP.subtract)
        xn = sb_x.tile([C, HW], d2, tag="xn")
        nc.vector.tensor_scalar(out=xn, in0=x_sb,
                                scalar1=aff[:, 0:1], scalar2=aff[:, 1:2],
                                op0=OP.mult, op1=OP.add)

        # ---------- Q, K, Vt ----------
        qp = ps_s.tile([C, HW], f32, tag="s")
        kp = ps_s.tile([C, HW], f32, tag="s")
        for n in range(2):
            sl = slice(n * NHALF, (n + 1) * NHALF)
            nc.tensor.matmul(qp[:, sl], wq_sb, xn[:, sl], start=True, stop=True)
            nc.tensor.matmul(kp[:, sl], wk_sb, xn[:, sl], start=True, stop=True)
        q_sb = sb_qk.tile([C, HW], d1, tag="q")
        k_sb = sb_qk.tile([C, HW], d1, tag="k")
        nc.vector.tensor_copy(out=q_sb, in_=qp)
        nc.vector.tensor_copy(out=k_sb, in_=kp)

        vp = ps_s.tile([C, HW], f32, tag="s")
        for j in range(8):
            nc.tensor.matmul(vp[:, j * 128:(j + 1) * 128],
                             xn[:, j * 128:(j + 1) * 128], wv_sb,
                             start=True, stop=True)
        vt = sb_qk.tile([C, 8, NH, 33], d1, tag="vt")
        nc.vector.tensor_copy(
            out=vt[:, :, :, 32:33],
            in_=ones32.rearrange("p (a b c) -> p a b c", a=8, b=NH))
        for j in range(8):
            src_v = vp[:, j * 128:(j + 1) * 128].rearrange(
                "p (h d) -> p h d", d=DH)
            nc.vector.tensor_copy(out=vt[:, j, :, 0:DH], in_=src_v)

        # ---------- attention (head pairs; SV = M=33 at base 0) ----------
        hun = sb_x.tile([C, HW], f32, tag="hun")
        zp = sb_sm.tile([C, NH, 8], f32, tag="zp")
        for pair in range(2):
            ha0 = ps_h.tile([33, HW], f32, tag="ha0")
            ha1 = ps_h.tile([33, HW], f32, tag="ha1")
            ha = {0: ha0, 1: ha1}
            for j in range(8):
                a_sb = sb_a.tile([C, 2, HW], d1, tag="a")
                for k in range(2):
                    h = 2 * pair + k
                    hp = slice(32 * h, 32 * h + 32)
                    sp = ps_s.tile([C, HW], f32, tag="s")
                    for n in range(2):
                        sl = slice(n * NHALF, (n + 1) * NHALF)
                        nc.tensor.matmul(
                            sp[:, sl],
                            k_sb[hp, j * 128:(j + 1) * 128],
                            q_sb[hp, sl],
                            start=True, stop=True,
                            tile_position=(32 * h, 0))
                    nc.scalar.activation(out=a_sb[:, k, :], in_=sp,
                                         func=AF.Exp, bias=0.0, scale=1.0)
                    for n in range(2):
                        sl = slice(n * NHALF, (n + 1) * NHALF)
                        nc.tensor.matmul(
                            ha[k][:, sl],
                            vt[:, j, h, :],
                            a_sb[:, k, sl],
                            start=(j == 0), stop=(j == 7))
            for k in range(2):
                h = 2 * pair + k
                hv = sb_sm.tile([33, HW], f32, tag="hv%d" % k)
                nc.vector.tensor_copy(out=hv, in_=ha[k])
                nc.sync.dma_start(out=zp[:, h, :], in_=hv[32:33, :])
                nc.sync.dma_start(out=hun[32 * h:32 * h + 32, :],
                                  in_=hv[0:32, :])

        # ----- 1/Z and normalize -----
        rp = sb_sm.tile([C, NH, 8], f32, tag="rp")
        nc.vector.reciprocal(out=rp, in_=zp)
        ral = sb_sm.tile([NH, HW], f32, tag="ral")
        for h in range(NH):
            nc.sync.dma_start(out=ral[h:h + 1, :], in_=rp[:, h, :])
        rb_ps = ps_s.tile([C, HW], f32, tag="s")
        for n in range(2):
            sl = slice(n * NHALF, (n + 1) * NHALF)
            nc.tensor.matmul(rb_ps[:, sl], ebc_sb, ral[:, sl],
                             start=True, stop=True)
        rb = sb_x.tile([C, HW], f32, tag="rb")
        nc.vector.tensor_copy(out=rb, in_=rb_ps)
        hn = sb_x.tile([C, HW], d1, tag="hn")
        nc.vector.tensor_mul(out=hn, in0=hun, in1=rb)

        # ---------- proj + bias + residual ----------
        pp = ps_s.tile([C, HW], f32, tag="s")
        for n in range(2):
            sl = slice(n * NHALF, (n + 1) * NHALF)
            nc.tensor.matmul(pp[:, sl], pjb_sb, ones_sb[:, sl],
                             start=True, stop=False, tile_position=(0, 0))
            nc.tensor.matmul(pp[:, sl], pj_sb, hn[:, sl],
                             start=False, stop=True)
        out_sb = sb_x.tile([C, HW], f32, tag="out")
        nc.vector.tensor_add(out=out_sb, in0=pp, in1=x_sb)
        nc.sync.dma_start(out=y_d[b], in_=out_sb)


def _get_nc():
    if "nc" not in _CACHE:
        _CACHE["nc"] = _build_nc()
    return _CACHE["nc"]


def _bf16_round(a):
    b = np.ascontiguousarray(a, np.float32).view(np.uint32)
    b = (b + 0x8000 - ((b >> 16) & 1)) & 0xFFFF0000
    return b.view(np.float32)


def _host_prep(inputs):
    x = np.ascontiguousarray(
        np.asarray(inputs["x"], np.float32).reshape(B, C, HW))
    qkv_w = np.asarray(inputs["qkv_w"], np.float32)
    proj_w = np.asarray(inputs["proj_w"], np.float32)
    proj_b = np.asarray(inputs["proj_b"], np.float32)
    norm_w = np.asarray(inputs["norm_w"], np.float32)
    norm_b = np.asarray(inputs["norm_b"], np.float32)

    w3 = qkv_w.reshape(NH, 3, DH, C)  # rows: h*96 + which*32 + d
    wq = w3[:, 0].reshape(C, C)
    wk = w3[:, 1].reshape(C, C)
    wv = w3[:, 2].reshape(C, C)
    wqT = np.ascontiguousarray((wq / 32.0).T)  # fold scale^2 = 1/dh
    wkT = np.ascontiguousarray(wk.T)
    wvT = np.ascontiguousarray(wv.T)
    if RLEVEL >= 2:
        wqT, wkT, wvT = _bf16_round(wqT), _bf16_round(wkT), _bf16_round(wvT)

    pjT = np.ascontiguousarray(proj_w.T)
    if RLEVEL >= 1:
        pjT = _bf16_round(pjT)

    g1 = np.zeros((C, GROUPS), np.float32)
    g1[np.arange(C), np.arange(C) // 4] = 0.25
    g2 = np.zeros((GROUPS, C), np.float32)
    g2[np.arange(C) // 4, np.arange(C)] = 1.0
    ebc = np.zeros((NH, C), np.float32)
    for h in range(NH):
        ebc[h, 32 * h:32 * h + 32] = 1.0

    params = dict(
        wqT=wqT, wkT=wkT, wvT=wvT, pjT=pjT,
        pjb=np.ascontiguousarray(proj_b[None, :]),
        nw=np.ascontiguousarray(norm_w[:, None]),
        nb=np.ascontiguousarray(norm_b[:, None]),
        g1=g1, g2=g2, ebc=ebc,
    )
    in_maps = []
    for i in range(NCORES):
        m = dict(params)
        m["x"] = np.ascontiguousarray(x[i * BPC:(i + 1) * BPC])
        in_maps.append(m)
    return in_maps


def kernel(**inputs):
    global LAST_RESULT
    from concourse.bass_utils import run_bass_kernel_spmd
    in_maps = _host_prep(inputs)
    nc = _get_nc()
    res = run_bass_kernel_spmd(nc, in_maps, list(range(NCORES)), trace=TRACE)
    LAST_RESULT = res
    y = np.concatenate([res.results[i]["y"] for i in range(NCORES)], axis=0)
    return y.reshape(B, C, 32, 32)


# revision 37
# speedup vs baseline: 1.0070x; 1.0070x over previous
"""AttentionBlock Trainium2 Bass kernel.

Full inputs -> shard batch over 8 NeuronCores (4 samples each) -> full output.

Per-sample on-core pipeline:
  x [C=128p, HW=1024f] -> groupnorm (bn_stats + PE group-reduce + affine)
  -> Q,K (per-head channel layout, scale^2 folded into Wq), V computed
     pre-transposed (Vt[s, (h,d)])
  -> S'[s,t] = K^T Q per head via K=32 matmuls row-tiled into PE quadrants;
     exp on ScalarE (no max subtraction: |S| < 1.3)
  -> hout via M=32 col-tiled matmuls (4 heads concurrent, standard channel
     order) + Z via M=1 ones-matmuls, accumulating over s-chunks in PSUM
  -> normalize with 1/Z (packed DVE reciprocal + PE broadcast)
  -> proj (+bias via K=1 ones matmul) + residual add -> y

float32r (single-pass fp32 matmul, bf16-rounded multiply) is used for the
qkv/attention/proj matmuls; groupnorm statistics stay exact fp32.
"""

import numpy as np
from contextlib import ExitStack

B, C, HW = 32, 128, 1024
NH, DH = 4, 32
GROUPS = 32
EPS = 1e-5
NCORES = 8
BPC = B // NCORES  # samples per core

_CACHE = {}
TRACE = False
LAST_RESULT = None
# 0 = all fp32; 1 = f32r attention+proj; 2 = + qkv/groupnorm-apply
RLEVEL = 2


def _patch_tile_waits(tile, mybir):
    """This walrus build encodes only one sync-wait slot per instruction;
    Tile can attach several. Split extra waits onto NoOps committed
    immediately before the instruction on the same engine queue
    (in-order => identical semantics)."""
    if getattr(tile.TileContext, "_mm_wait_patched", False):
        return
    orig = tile.TileContext._commit_instruction

    def patched(self, inst, lazy_reg_writes=True):
        si = getattr(inst, "sync_info", None)
        if (not isinstance(inst, mybir.InstNoOp) and si is not None
                and si.on_wait and len(si.on_wait) > 1):
            waits = list(si.on_wait)
            for w in waits[:-1]:
                nop = mybir.InstNoOp(
                    name=self.nc.get_next_instruction_name(),
                    engine=inst.engine,
                    bass_nofuse=True,
                    sync_info=mybir.SyncInfo(on_wait=[w], on_update=[]),
                )
                orig(self, nop, lazy_reg_writes=False)
            inst.sync_info = mybir.SyncInfo(
                on_wait=[waits[-1]], on_update=list(si.on_update))
        return orig(self, inst, lazy_reg_writes)

    tile.TileContext._commit_instruction = patched

    def patched_drain(self, tick_clock, wait_clock):
        # Collect end-of-kernel waits, then hand them out one per SP nop
        # (the drain keeps none); nops precede the teardown barrier on the
        # same queue, so semantics are preserved.
        self.nc.sync.drain()
        sink = self.nc.sync.nop(nofuse=True)
        wait_clock.add_sem_waits(
            sink.ins, tile.ScopedClock({None: tick_clock.global_clock}))
        si = sink.ins.sync_info
        waits = list(si.on_wait) if si and si.on_wait else []
        if len(waits) > 1:
            sink.ins.sync_info = mybir.SyncInfo(
                on_wait=[waits[0]], on_update=list(si.on_update))
            for w in waits[1:]:
                extra = self.nc.sync.nop(nofuse=True)
                extra.ins.sync_info = mybir.SyncInfo(on_wait=[w], on_update=[])

        self.nc.all_engine_barrier()
        assert self.sems is not None
        popped = self.nc._tile_sem_poison_stack.pop()
        assert popped is self._sem_poison
        self.nc.clear_and_free_semaphores(list(self.sems.allocated().values()))
        self.nc.all_engine_barrier()

    tile.TileContext._drain_and_barrier = patched_drain
    tile.TileContext._mm_wait_patched = True


def _build_nc():
    import concourse.bass as bass
    import concourse.tile as tile
    from concourse import mybir

    _patch_tile_waits(tile, mybir)

    f32 = mybir.dt.float32
    d1 = mybir.dt.float32r if RLEVEL >= 1 else f32
    d2 = mybir.dt.float32r if RLEVEL >= 2 else f32
    nc = bass.Bass()

    x_d = nc.dram_tensor("x", [BPC, C, HW], f32, kind="ExternalInput")
    wq_d = nc.dram_tensor("wqT", [C, C], d2, kind="ExternalInput")
    wk_d = nc.dram_tensor("wkT", [C, C], d2, kind="ExternalInput")
    wv_d = nc.dram_tensor("wvT", [C, C], d2, kind="ExternalInput")
    pjT_d = nc.dram_tensor("pjT", [C, C], d1, kind="ExternalInput")
    pjb_d = nc.dram_tensor("pjb", [1, C], f32, kind="ExternalInput")
    nw_d = nc.dram_tensor("nw", [C, 1], f32, kind="ExternalInput")
    nb_d = nc.dram_tensor("nb", [C, 1], f32, kind="ExternalInput")
    g1_d = nc.dram_tensor("g1", [C, GROUPS], f32, kind="ExternalInput")
    g2_d = nc.dram_tensor("g2", [GROUPS, C], f32, kind="ExternalInput")
    ebc_d = nc.dram_tensor("ebc", [NH, C], f32, kind="ExternalInput")
    y_d = nc.dram_tensor("y", [BPC, C, HW], f32, kind="ExternalOutput")

    with tile.TileContext(nc) as tc:
        with ExitStack() as ctx:
            _body(ctx, tc, mybir, bass, d1, d2,
                  x_d, wq_d, wk_d, wv_d, pjT_d, pjb_d, nw_d, nb_d,
                  g1_d, g2_d, ebc_d, y_d)
    return nc


def _body(ctx, tc, mybir, bass, d1, d2,
          x_d, wq_d, wk_d, wv_d, pjT_d, pjb_d, nw_d, nb_d, g1_d, g2_d,
          ebc_d, y_d):
    nc = tc.nc
    f32 = mybir.dt.float32
    AF = mybir.ActivationFunctionType
    OP = mybir.AluOpType

    const = ctx.enter_context(tc.tile_pool(name="const", bufs=1))
    sb_x = ctx.enter_context(tc.tile_pool(name="sb_x", bufs=2))
    sb_qk = ctx.enter_context(tc.tile_pool(name="sb_qk", bufs=2))
    sb_a = ctx.enter_context(tc.tile_pool(name="sb_a", bufs=3))
    sb_sm = ctx.enter_context(tc.tile_pool(name="sb_sm", bufs=4))
    # PSUM budget: ps_s 2x[128,1024] (4 banks) + ps_h tags h,z (4 banks) = 8
    ps_s = ctx.enter_context(tc.tile_pool(name="ps_s", bufs=2, space="PSUM"))
    ps_h = ctx.enter_context(tc.tile_pool(name="ps_h", bufs=1, space="PSUM"))

    # ---- constants ----
    wq_sb = const.tile([C, C], d2, tag="wq")
    wk_sb = const.tile([C, C], d2, tag="wk")
    wv_sb = const.tile([C, C], d2, tag="wv")
    pj_sb = const.tile([C, C], d1, tag="pj")
    pjb_sb = const.tile([1, C], f32, tag="pjb")
    nw_sb = const.tile([C, 1], f32, tag="nw")
    nb_sb = const.tile([C, 1], f32, tag="nb")
    g1_sb = const.tile([C, GROUPS], f32, tag="g1")
    g2_sb = const.tile([GROUPS, C], f32, tag="g2")
    ebc_sb = const.tile([NH, C], f32, tag="ebc")
    for dst, src in ((wq_sb, wq_d), (wk_sb, wk_d), (wv_sb, wv_d),
                     (pj_sb, pjT_d), (pjb_sb, pjb_d), (nw_sb, nw_d),
                     (nb_sb, nb_d), (g1_sb, g1_d), (g2_sb, g2_d),
                     (ebc_sb, ebc_d)):
        nc.sync.dma_start(out=dst, in_=src[:])
    ones_sb = const.tile([1, HW], f32, tag="ones")
    nc.vector.memset(ones_sb, 1.0)
    eps_sb = const.tile([GROUPS, 1], f32, tag="eps")
    nc.vector.memset(eps_sb, EPS)
    ones32 = const.tile([C, DH], f32, tag="ones32")
    nc.vector.memset(ones32, 1.0)


    NHALF = HW // 2  # 512

    for b in range(BPC):
        # ---------- load ----------
        x_sb = sb_x.tile([C, HW], f32, tag="x")
        nc.sync.dma_start(out=x_sb, in_=x_d[b])

        # ---------- groupnorm ----------
        st6 = sb_sm.tile([C, 2, 6], f32, tag="st6")
        nc.vector.bn_stats(out=st6[:, 0, :], in_=x_sb[:, 0:512])
        nc.vector.bn_stats(out=st6[:, 1, :], in_=x_sb[:, 512:1024])
        mv = sb_sm.tile([C, 2], f32, tag="mv")
        nc.vector.bn_aggr(out=mv, in_=st6)
        # s2 = [mean_c, mean_c^2 + var_c]
        s2 = sb_sm.tile([C, 2], f32, tag="s2")
        nc.vector.tensor_copy(out=s2[:, 0:1], in_=mv[:, 0:1])
        nc.vector.tensor_mul(out=s2[:, 1:2], in0=mv[:, 0:1], in1=mv[:, 0:1])
        nc.vector.tensor_add(out=s2[:, 1:2], in0=s2[:, 1:2], in1=mv[:, 1:2])
        # group reduce: [32, 2] = (g1/4)^T @ s2
        gp = ps_s.tile([C, HW], f32, tag="s")
        nc.tensor.matmul(gp[0:GROUPS, 0:2], g1_sb, s2, start=True, stop=True)
        gs = sb_sm.tile([GROUPS, 2], f32, tag="gs")
        nc.vector.tensor_copy(out=gs, in_=gp[0:GROUPS, 0:2])
        # vv = [mu_g, rstd_g]; rstd = exp(-0.5*ln(var+eps)) (same ACT set)
        vv = sb_sm.tile([GROUPS, 2], f32, tag="vv")
        nc.vector.tensor_mul(out=vv[:, 0:1], in0=gs[:, 0:1], in1=gs[:, 0:1])
        nc.vector.tensor_tensor(out=vv[:, 1:2], in0=gs[:, 1:2], in1=vv[:, 0:1],
                                op=OP.subtract)
        nc.scalar.activation(out=vv[:, 1:2], in_=vv[:, 1:2], func=AF.Ln,
                             bias=eps_sb, scale=1.0)
        nc.scalar.activation(out=vv[:, 1:2], in_=vv[:, 1:2], func=AF.Exp,
                             bias=0.0, scale=-0.5)
        nc.vector.tensor_copy(out=vv[:, 0:1], in_=gs[:, 0:1])
        # broadcast to channels: bc[c, 0:2] = [mu_c, rstd_c]
        bc = ps_s.tile([C, HW], f32, tag="s")
        nc.tensor.matmul(bc[0:C, 0:2], g2_sb, vv, start=True, stop=True)
        aff = sb_sm.tile([C, 2], f32, tag="aff")
        nc.vector.tensor_mul(out=aff[:, 0:1], in0=nw_sb, in1=bc[:, 1:2])
        nc.vector.tensor_mul(out=aff[:, 1:2], in0=bc[:, 0:1], in1=aff[:, 0:1])
        nc.vector.tensor_tensor(out=aff[:, 1:2], in0=nb_sb, in1=aff[:, 1:2],
                                op=OP.subtract)
        xn = sb_x.tile([C, HW], d2, tag="xn")
        nc.vector.tensor_scalar(out=xn, in0=x_sb,
                                scalar1=aff[:, 0:1], scalar2=aff[:, 1:2],
                                op0=OP.mult, op1=OP.add)

        # ---------- Q, K, Vt ----------
        qp = ps_s.tile([C, HW], f32, tag="s")
        kp = ps_s.tile([C, HW], f32, tag="s")
        for n in range(2):
            sl = slice(n * NHALF, (n + 1) * NHALF)
            nc.tensor.matmul(qp[:, sl], wq_sb, xn[:, sl], start=True, stop=True)
            nc.tensor.matmul(kp[:, sl], wk_sb, xn[:, sl], start=True, stop=True)
        q_sb = sb_qk.tile([C, HW], d1, tag="q")
        k_sb = sb_qk.tile([C, HW], d1, tag="k")
        nc.vector.tensor_copy(out=q_sb, in_=qp)
        nc.vector.tensor_copy(out=k_sb, in_=kp)

        vp = ps_s.tile([C, HW], f32, tag="s")
        for j in range(8):
            nc.tensor.matmul(vp[:, j * 128:(j + 1) * 128],
                             xn[:, j * 128:(j + 1) * 128], wv_sb,
                             start=True, stop=True)
        vt = sb_qk.tile([C, 8, NH, 33], d1, tag="vt")
        nc.vector.tensor_copy(
            out=vt[:, :, :, 32:33],
            in_=ones32.rearrange("p (a b c) -> p a b c", a=8, b=NH))
        for j in range(8):
            src_v = vp[:, j * 128:(j + 1) * 128].rearrange(
                "p (h d) -> p h d", d=DH)
            nc.vector.tensor_copy(out=vt[:, j, :, 0:DH], in_=src_v)

        # ---------- attention (head pairs; SV = M=33 at base 0) ----------
        hun = sb_x.tile([C, HW], f32, tag="hun")
        zp = sb_sm.tile([C, NH, 8], f32, tag="zp")
        for pair in range(2):
            ha0 = ps_h.tile([33, HW], f32, tag="ha0")
            ha1 = ps_h.tile([33, HW], f32, tag="ha1")
            ha = {0: ha0, 1: ha1}
            for j in range(8):
                a_sb = sb_a.tile([C, 2, HW], d1, tag="a")
                sp0 = ps_s.tile([C, HW], f32, tag="s")
                sp1 = ps_s.tile([C, HW], f32, tag="s")
                sps = (sp0, sp1)
                # S' for both heads first, head-interleaved so the two row
                # groups stream concurrently in the PE array
                for n in range(2):
                    sl = slice(n * NHALF, (n + 1) * NHALF)
                    for k in range(2):
                        h = 2 * pair + k
                        hp = slice(32 * h, 32 * h + 32)
                        nc.tensor.matmul(
                            sps[k][:, sl],
                            k_sb[hp, j * 128:(j + 1) * 128],
                            q_sb[hp, sl],
                            start=True, stop=True,
                            tile_position=(32 * h, 0))
                for k in range(2):
                    nc.scalar.activation(out=a_sb[:, k, :], in_=sps[k],
                                         func=AF.Exp, bias=0.0, scale=1.0)
                for k in range(2):
                    h = 2 * pair + k
                    for n in range(2):
                        sl = slice(n * NHALF, (n + 1) * NHALF)
                        nc.tensor.matmul(
                            ha[k][:, sl],
                            vt[:, j, h, :],
                            a_sb[:, k, sl],
                            start=(j == 0), stop=(j == 7))
            for k in range(2):
                h = 2 * pair + k
                hv = sb_sm.tile([33, HW], f32, tag="hv%d" % k)
                nc.vector.tensor_copy(out=hv, in_=ha[k])
                nc.sync.dma_start(out=zp[:, h, :], in_=hv[32:33, :])
                nc.sync.dma_start(out=hun[32 * h:32 * h + 32, :],
                                  in_=hv[0:32, :])

        # ----- 1/Z and normalize -----
        rp = sb_sm.tile([C, NH, 8], f32, tag="rp")
        nc.vector.reciprocal(out=rp, in_=zp)
        ral = sb_sm.tile([NH, HW], f32, tag="ral")
        for h in range(NH):
            nc.sync.dma_start(out=ral[h:h + 1, :], in_=rp[:, h, :])
        rb_ps = ps_s.tile([C, HW], f32, tag="s")
        for n in range(2):
            sl = slice(n * NHALF, (n + 1) * NHALF)
            nc.tensor.matmul(rb_ps[:, sl], ebc_sb, ral[:, sl],
                             start=True, stop=True)
        rb = sb_x.tile([C, HW], f32, tag="rb")
        nc.vector.tensor_copy(out=rb, in_=rb_ps)
        hn = sb_x.tile([C, HW], d1, tag="hn")
        nc.vector.tensor_mul(out=hn, in0=hun, in1=rb)

        # ---------- proj + bias + residual ----------
        pp = ps_s.tile([C, HW], f32, tag="s")
        for n in range(2):
            sl = slice(n * NHALF, (n + 1) * NHALF)
            nc.tensor.matmul(pp[:, sl], pjb_sb, ones_sb[:, sl],
                             start=True, stop=False, tile_position=(0, 0))
            nc.tensor.matmul(pp[:, sl], pj_sb, hn[:, sl],
                             start=False, stop=True)
        out_sb = sb_x.tile([C, HW], f32, tag="out")
        nc.vector.tensor_add(out=out_sb, in0=pp, in1=x_sb)
        nc.sync.dma_start(out=y_d[b], in_=out_sb)


def _get_nc():
    if "nc" not in _CACHE:
        _CACHE["nc"] = _build_nc()
    return _CACHE["nc"]


def _bf16_round(a):
    b = np.ascontiguousarray(a, np.float32).view(np.uint32)
    b = (b + 0x8000 - ((b >> 16) & 1)) & 0xFFFF0000
    return b.view(np.float32)


def _host_prep(inputs):
    x = np.ascontiguousarray(
        np.asarray(inputs["x"], np.float32).reshape(B, C, HW))
    qkv_w = np.asarray(inputs["qkv_w"], np.float32)
    proj_w = np.asarray(inputs["proj_w"], np.float32)
    proj_b = np.asarray(inputs["proj_b"], np.float32)
    norm_w = np.asarray(inputs["norm_w"], np.float32)
    norm_b = np.asarray(inputs["norm_b"], np.float32)

    w3 = qkv_w.reshape(NH, 3, DH, C)  # rows: h*96 + which*32 + d
    wq = w3[:, 0].reshape(C, C)
    wk = w3[:, 1].reshape(C, C)
    wv = w3[:, 2].reshape(C, C)
    wqT = np.ascontiguousarray((wq / 32.0).T)  # fold scale^2 = 1/dh
    wkT = np.ascontiguousarray(wk.T)
    wvT = np.ascontiguousarray(wv.T)
    if RLEVEL >= 2:
        wqT, wkT, wvT = _bf16_round(wqT), _bf16_round(wkT), _bf16_round(wvT)

    pjT = np.ascontiguousarray(proj_w.T)
    if RLEVEL >= 1:
        pjT = _bf16_round(pjT)

    g1 = np.zeros((C, GROUPS), np.float32)
    g1[np.arange(C), np.arange(C) // 4] = 0.25
    g2 = np.zeros((GROUPS, C), np.float32)
    g2[np.arange(C) // 4, np.arange(C)] = 1.0
    ebc = np.zeros((NH, C), np.float32)
    for h in range(NH):
        ebc[h, 32 * h:32 * h + 32] = 1.0

    params = dict(
        wqT=wqT, wkT=wkT, wvT=wvT, pjT=pjT,
        pjb=np.ascontiguousarray(proj_b[None, :]),
        nw=np.ascontiguousarray(norm_w[:, None]),
        nb=np.ascontiguousarray(norm_b[:, None]),
        g1=g1, g2=g2, ebc=ebc,
    )
    in_maps = []
    for i in range(NCORES):
        m = dict(params)
        m["x"] = np.ascontiguousarray(x[i * BPC:(i + 1) * BPC])
        in_maps.append(m)
    return in_maps


def kernel(**inputs):
    global LAST_RESULT
    from concourse.bass_utils import run_bass_kernel_spmd
    in_maps = _host_prep(inputs)
    nc = _get_nc()
    res = run_bass_kernel_spmd(nc, in_maps, list(range(NCORES)), trace=TRACE)
    LAST_RESULT = res
    y = np.concatenate([res.results[i]["y"] for i in range(NCORES)], axis=0)
    return y.reshape(B, C, 32, 32)


# revision 38
# speedup vs baseline: 1.0938x; 1.0861x over previous
"""AttentionBlock Trainium2 Bass kernel.

Full inputs -> shard batch over 8 NeuronCores (4 samples each) -> full output.

Per-sample on-core pipeline:
  x [C=128p, HW=1024f] -> groupnorm (bn_stats + PE group-reduce + affine)
  -> Q,K (per-head channel layout, scale^2 folded into Wq), V computed
     pre-transposed (Vt[s, (h,d)])
  -> S'[s,t] = K^T Q per head via K=32 matmuls row-tiled into PE quadrants;
     exp on ScalarE (no max subtraction: |S| < 1.3)
  -> hout via M=32 col-tiled matmuls (4 heads concurrent, standard channel
     order) + Z via M=1 ones-matmuls, accumulating over s-chunks in PSUM
  -> normalize with 1/Z (packed DVE reciprocal + PE broadcast)
  -> proj (+bias via K=1 ones matmul) + residual add -> y

float32r (single-pass fp32 matmul, bf16-rounded multiply) is used for the
qkv/attention/proj matmuls; groupnorm statistics stay exact fp32.
"""

import numpy as np
from contextlib import ExitStack

B, C, HW = 32, 128, 1024
NH, DH = 4, 32
GROUPS = 32
EPS = 1e-5
NCORES = 8
BPC = B // NCORES  # samples per core

_CACHE = {}
TRACE = False
LAST_RESULT = None
# matmul precision for qkv/attention/proj (groupnorm stats stay fp32):
#   "f32"  - exact, 4 cyc/col
#   "f32r" - stationary bf16-rounded, moving fp32, 2 cyc/col (~1.6e-4 rel)
#   "bf16" - both operands bf16, 1 cyc/col (~3e-3 rel; gate is 2e-2)
PREC = "bf16"
RLEVEL = 2  # legacy flag: host-side pre-round of weights for f32r


def _patch_tile_waits(tile, mybir):
    """This walrus build encodes only one sync-wait slot per instruction;
    Tile can attach several. Split extra waits onto NoOps committed
    immediately before the instruction on the same engine queue
    (in-order => identical semantics)."""
    if getattr(tile.TileContext, "_mm_wait_patched", False):
        return
    orig = tile.TileContext._commit_instruction

    def patched(self, inst, lazy_reg_writes=True):
        si = getattr(inst, "sync_info", None)
        if (not isinstance(inst, mybir.InstNoOp) and si is not None
                and si.on_wait and len(si.on_wait) > 1):
            waits = list(si.on_wait)
            for w in waits[:-1]:
                nop = mybir.InstNoOp(
                    name=self.nc.get_next_instruction_name(),
                    engine=inst.engine,
                    bass_nofuse=True,
                    sync_info=mybir.SyncInfo(on_wait=[w], on_update=[]),
                )
                orig(self, nop, lazy_reg_writes=False)
            inst.sync_info = mybir.SyncInfo(
                on_wait=[waits[-1]], on_update=list(si.on_update))
        return orig(self, inst, lazy_reg_writes)

    tile.TileContext._commit_instruction = patched

    def patched_drain(self, tick_clock, wait_clock):
        # Collect end-of-kernel waits, then hand them out one per SP nop
        # (the drain keeps none); nops precede the teardown barrier on the
        # same queue, so semantics are preserved.
        self.nc.sync.drain()
        sink = self.nc.sync.nop(nofuse=True)
        wait_clock.add_sem_waits(
            sink.ins, tile.ScopedClock({None: tick_clock.global_clock}))
        si = sink.ins.sync_info
        waits = list(si.on_wait) if si and si.on_wait else []
        if len(waits) > 1:
            sink.ins.sync_info = mybir.SyncInfo(
                on_wait=[waits[0]], on_update=list(si.on_update))
            for w in waits[1:]:
                extra = self.nc.sync.nop(nofuse=True)
                extra.ins.sync_info = mybir.SyncInfo(on_wait=[w], on_update=[])

        self.nc.all_engine_barrier()
        assert self.sems is not None
        popped = self.nc._tile_sem_poison_stack.pop()
        assert popped is self._sem_poison
        self.nc.clear_and_free_semaphores(list(self.sems.allocated().values()))
        self.nc.all_engine_barrier()

    tile.TileContext._drain_and_barrier = patched_drain
    tile.TileContext._mm_wait_patched = True


def _build_nc():
    import concourse.bass as bass
    import concourse.tile as tile
    from concourse import mybir

    _patch_tile_waits(tile, mybir)

    f32 = mybir.dt.float32
    _prec = {"f32": f32, "f32r": mybir.dt.float32r,
             "bf16": mybir.dt.bfloat16}[PREC]
    d1 = d2 = _prec
    nc = bass.Bass()

    x_d = nc.dram_tensor("x", [BPC, C, HW], f32, kind="ExternalInput")
    wq_d = nc.dram_tensor("wqT", [C, C], d2, kind="ExternalInput")
    wk_d = nc.dram_tensor("wkT", [C, C], d2, kind="ExternalInput")
    wv_d = nc.dram_tensor("wvT", [C, C], d2, kind="ExternalInput")
    pjT_d = nc.dram_tensor("pjT", [C, C], d1, kind="ExternalInput")
    pjb_d = nc.dram_tensor("pjb", [1, C], f32, kind="ExternalInput")
    nw_d = nc.dram_tensor("nw", [C, 1], f32, kind="ExternalInput")
    nb_d = nc.dram_tensor("nb", [C, 1], f32, kind="ExternalInput")
    g1_d = nc.dram_tensor("g1", [C, GROUPS], f32, kind="ExternalInput")
    g2_d = nc.dram_tensor("g2", [GROUPS, C], f32, kind="ExternalInput")
    ebc_d = nc.dram_tensor("ebc", [NH, C], f32, kind="ExternalInput")
    y_d = nc.dram_tensor("y", [BPC, C, HW], f32, kind="ExternalOutput")

    with tile.TileContext(nc) as tc:
        with ExitStack() as ctx:
            _body(ctx, tc, mybir, bass, d1, d2,
                  x_d, wq_d, wk_d, wv_d, pjT_d, pjb_d, nw_d, nb_d,
                  g1_d, g2_d, ebc_d, y_d)
    return nc


def _body(ctx, tc, mybir, bass, d1, d2,
          x_d, wq_d, wk_d, wv_d, pjT_d, pjb_d, nw_d, nb_d, g1_d, g2_d,
          ebc_d, y_d):
    nc = tc.nc
    f32 = mybir.dt.float32
    AF = mybir.ActivationFunctionType
    OP = mybir.AluOpType

    const = ctx.enter_context(tc.tile_pool(name="const", bufs=1))
    sb_x = ctx.enter_context(tc.tile_pool(name="sb_x", bufs=2))
    sb_qk = ctx.enter_context(tc.tile_pool(name="sb_qk", bufs=2))
    sb_a = ctx.enter_context(tc.tile_pool(name="sb_a", bufs=3))
    sb_sm = ctx.enter_context(tc.tile_pool(name="sb_sm", bufs=4))
    # PSUM budget: ps_s 2x[128,1024] (4 banks) + ps_h tags h,z (4 banks) = 8
    ps_s = ctx.enter_context(tc.tile_pool(name="ps_s", bufs=2, space="PSUM"))
    ps_h = ctx.enter_context(tc.tile_pool(name="ps_h", bufs=1, space="PSUM"))

    # ---- constants ----
    wq_sb = const.tile([C, C], d2, tag="wq")
    wk_sb = const.tile([C, C], d2, tag="wk")
    wv_sb = const.tile([C, C], d2, tag="wv")
    pj_sb = const.tile([C, C], d1, tag="pj")
    pjb_sb = const.tile([1, C], f32, tag="pjb")
    nw_sb = const.tile([C, 1], f32, tag="nw")
    nb_sb = const.tile([C, 1], f32, tag="nb")
    g1_sb = const.tile([C, GROUPS], f32, tag="g1")
    g2_sb = const.tile([GROUPS, C], f32, tag="g2")
    ebc_sb = const.tile([NH, C], f32, tag="ebc")
    for dst, src in ((wq_sb, wq_d), (wk_sb, wk_d), (wv_sb, wv_d),
                     (pj_sb, pjT_d), (pjb_sb, pjb_d), (nw_sb, nw_d),
                     (nb_sb, nb_d), (g1_sb, g1_d), (g2_sb, g2_d),
                     (ebc_sb, ebc_d)):
        nc.sync.dma_start(out=dst, in_=src[:])
    ones_sb = const.tile([1, HW], f32, tag="ones")
    nc.vector.memset(ones_sb, 1.0)
    eps_sb = const.tile([GROUPS, 1], f32, tag="eps")
    nc.vector.memset(eps_sb, EPS)
    ones32 = const.tile([C, DH], f32, tag="ones32")
    nc.vector.memset(ones32, 1.0)


    NHALF = HW // 2  # 512

    for b in range(BPC):
        # ---------- load ----------
        x_sb = sb_x.tile([C, HW], f32, tag="x")
        nc.sync.dma_start(out=x_sb, in_=x_d[b])

        # ---------- groupnorm ----------
        st6 = sb_sm.tile([C, 2, 6], f32, tag="st6")
        nc.vector.bn_stats(out=st6[:, 0, :], in_=x_sb[:, 0:512])
        nc.vector.bn_stats(out=st6[:, 1, :], in_=x_sb[:, 512:1024])
        mv = sb_sm.tile([C, 2], f32, tag="mv")
        nc.vector.bn_aggr(out=mv, in_=st6)
        # s2 = [mean_c, mean_c^2 + var_c]
        s2 = sb_sm.tile([C, 2], f32, tag="s2")
        nc.vector.tensor_copy(out=s2[:, 0:1], in_=mv[:, 0:1])
        nc.vector.tensor_mul(out=s2[:, 1:2], in0=mv[:, 0:1], in1=mv[:, 0:1])
        nc.vector.tensor_add(out=s2[:, 1:2], in0=s2[:, 1:2], in1=mv[:, 1:2])
        # group reduce: [32, 2] = (g1/4)^T @ s2
        gp = ps_s.tile([C, HW], f32, tag="s")
        nc.tensor.matmul(gp[0:GROUPS, 0:2], g1_sb, s2, start=True, stop=True)
        gs = sb_sm.tile([GROUPS, 2], f32, tag="gs")
        nc.vector.tensor_copy(out=gs, in_=gp[0:GROUPS, 0:2])
        # vv = [mu_g, rstd_g]; rstd = exp(-0.5*ln(var+eps)) (same ACT set)
        vv = sb_sm.tile([GROUPS, 2], f32, tag="vv")
        nc.vector.tensor_mul(out=vv[:, 0:1], in0=gs[:, 0:1], in1=gs[:, 0:1])
        nc.vector.tensor_tensor(out=vv[:, 1:2], in0=gs[:, 1:2], in1=vv[:, 0:1],
                                op=OP.subtract)
        nc.scalar.activation(out=vv[:, 1:2], in_=vv[:, 1:2], func=AF.Ln,
                             bias=eps_sb, scale=1.0)
        nc.scalar.activation(out=vv[:, 1:2], in_=vv[:, 1:2], func=AF.Exp,
                             bias=0.0, scale=-0.5)
        nc.vector.tensor_copy(out=vv[:, 0:1], in_=gs[:, 0:1])
        # broadcast to channels: bc[c, 0:2] = [mu_c, rstd_c]
        bc = ps_s.tile([C, HW], f32, tag="s")
        nc.tensor.matmul(bc[0:C, 0:2], g2_sb, vv, start=True, stop=True)
        aff = sb_sm.tile([C, 2], f32, tag="aff")
        nc.vector.tensor_mul(out=aff[:, 0:1], in0=nw_sb, in1=bc[:, 1:2])
        nc.vector.tensor_mul(out=aff[:, 1:2], in0=bc[:, 0:1], in1=aff[:, 0:1])
        nc.vector.tensor_tensor(out=aff[:, 1:2], in0=nb_sb, in1=aff[:, 1:2],
                                op=OP.subtract)
        xn = sb_x.tile([C, HW], d2, tag="xn")
        nc.vector.tensor_scalar(out=xn, in0=x_sb,
                                scalar1=aff[:, 0:1], scalar2=aff[:, 1:2],
                                op0=OP.mult, op1=OP.add)

        # ---------- Q, K, Vt ----------
        qp = ps_s.tile([C, HW], f32, tag="s")
        kp = ps_s.tile([C, HW], f32, tag="s")
        for n in range(2):
            sl = slice(n * NHALF, (n + 1) * NHALF)
            nc.tensor.matmul(qp[:, sl], wq_sb, xn[:, sl], start=True, stop=True)
            nc.tensor.matmul(kp[:, sl], wk_sb, xn[:, sl], start=True, stop=True)
        q_sb = sb_qk.tile([C, HW], d1, tag="q")
        k_sb = sb_qk.tile([C, HW], d1, tag="k")
        nc.vector.tensor_copy(out=q_sb, in_=qp)
        nc.vector.tensor_copy(out=k_sb, in_=kp)

        vp = ps_s.tile([C, HW], f32, tag="s")
        for j in range(8):
            nc.tensor.matmul(vp[:, j * 128:(j + 1) * 128],
                             xn[:, j * 128:(j + 1) * 128], wv_sb,
                             start=True, stop=True)
        vt = sb_qk.tile([C, 8, NH, 33], d1, tag="vt")
        nc.vector.tensor_copy(
            out=vt[:, :, :, 32:33],
            in_=ones32.rearrange("p (a b c) -> p a b c", a=8, b=NH))
        for j in range(8):
            src_v = vp[:, j * 128:(j + 1) * 128].rearrange(
                "p (h d) -> p h d", d=DH)
            nc.vector.tensor_copy(out=vt[:, j, :, 0:DH], in_=src_v)

        # ---------- attention (head pairs; SV = M=33 at base 0) ----------
        hun = sb_x.tile([C, HW], f32, tag="hun")
        zp = sb_sm.tile([C, NH, 8], f32, tag="zp")
        for pair in range(2):
            ha0 = ps_h.tile([33, HW], f32, tag="ha0")
            ha1 = ps_h.tile([33, HW], f32, tag="ha1")
            ha = {0: ha0, 1: ha1}
            for j in range(8):
                a_sb = sb_a.tile([C, 2, HW], d1, tag="a")
                sp0 = ps_s.tile([C, HW], f32, tag="s")
                sp1 = ps_s.tile([C, HW], f32, tag="s")
                sps = (sp0, sp1)
                # S' for both heads first, head-interleaved so the two row
                # groups stream concurrently in the PE array
                for n in range(2):
                    sl = slice(n * NHALF, (n + 1) * NHALF)
                    for k in range(2):
                        h = 2 * pair + k
                        hp = slice(32 * h, 32 * h + 32)
                        nc.tensor.matmul(
                            sps[k][:, sl],
                            k_sb[hp, j * 128:(j + 1) * 128],
                            q_sb[hp, sl],
                            start=True, stop=True,
                            tile_position=(32 * h, 0))
                for k in range(2):
                    nc.scalar.activation(out=a_sb[:, k, :], in_=sps[k],
                                         func=AF.Exp, bias=0.0, scale=1.0)
                for k in range(2):
                    h = 2 * pair + k
                    for n in range(2):
                        sl = slice(n * NHALF, (n + 1) * NHALF)
                        nc.tensor.matmul(
                            ha[k][:, sl],
                            vt[:, j, h, :],
                            a_sb[:, k, sl],
                            start=(j == 0), stop=(j == 7))
            for k in range(2):
                h = 2 * pair + k
                hv = sb_sm.tile([33, HW], f32, tag="hv%d" % k)
                nc.vector.tensor_copy(out=hv, in_=ha[k])
                nc.sync.dma_start(out=zp[:, h, :], in_=hv[32:33, :])
                nc.sync.dma_start(out=hun[32 * h:32 * h + 32, :],
                                  in_=hv[0:32, :])

        # ----- 1/Z and normalize -----
        rp = sb_sm.tile([C, NH, 8], f32, tag="rp")
        nc.vector.reciprocal(out=rp, in_=zp)
        ral = sb_sm.tile([NH, HW], f32, tag="ral")
        for h in range(NH):
            nc.sync.dma_start(out=ral[h:h + 1, :], in_=rp[:, h, :])
        rb_ps = ps_s.tile([C, HW], f32, tag="s")
        for n in range(2):
            sl = slice(n * NHALF, (n + 1) * NHALF)
            nc.tensor.matmul(rb_ps[:, sl], ebc_sb, ral[:, sl],
                             start=True, stop=True)
        rb = sb_x.tile([C, HW], f32, tag="rb")
        nc.vector.tensor_copy(out=rb, in_=rb_ps)
        hn = sb_x.tile([C, HW], d1, tag="hn")
        nc.vector.tensor_mul(out=hn, in0=hun, in1=rb)

        # ---------- proj + bias + residual ----------
        pp = ps_s.tile([C, HW], f32, tag="s")
        for n in range(2):
            sl = slice(n * NHALF, (n + 1) * NHALF)
            nc.tensor.matmul(pp[:, sl], pjb_sb, ones_sb[:, sl],
                             start=True, stop=False, tile_position=(0, 0))
            nc.tensor.matmul(pp[:, sl], pj_sb, hn[:, sl],
                             start=False, stop=True)
        out_sb = sb_x.tile([C, HW], f32, tag="out")
        nc.vector.tensor_add(out=out_sb, in0=pp, in1=x_sb)
        nc.sync.dma_start(out=y_d[b], in_=out_sb)


def _get_nc():
    if "nc" not in _CACHE:
        _CACHE["nc"] = _build_nc()
    return _CACHE["nc"]


def _bf16_round(a):
    b = np.ascontiguousarray(a, np.float32).view(np.uint32)
    b = (b + 0x8000 - ((b >> 16) & 1)) & 0xFFFF0000
    return b.view(np.float32)


def _host_prep(inputs):
    x = np.ascontiguousarray(
        np.asarray(inputs["x"], np.float32).reshape(B, C, HW))
    qkv_w = np.asarray(inputs["qkv_w"], np.float32)
    proj_w = np.asarray(inputs["proj_w"], np.float32)
    proj_b = np.asarray(inputs["proj_b"], np.float32)
    norm_w = np.asarray(inputs["norm_w"], np.float32)
    norm_b = np.asarray(inputs["norm_b"], np.float32)

    w3 = qkv_w.reshape(NH, 3, DH, C)  # rows: h*96 + which*32 + d
    wq = w3[:, 0].reshape(C, C)
    wk = w3[:, 1].reshape(C, C)
    wv = w3[:, 2].reshape(C, C)
    wqT = np.ascontiguousarray((wq / 32.0).T)  # fold scale^2 = 1/dh
    wkT = np.ascontiguousarray(wk.T)
    wvT = np.ascontiguousarray(wv.T)
    pjT = np.ascontiguousarray(proj_w.T)
    if PREC == "f32r":
        wqT, wkT, wvT = _bf16_round(wqT), _bf16_round(wkT), _bf16_round(wvT)
        pjT = _bf16_round(pjT)
    elif PREC == "bf16":
        import ml_dtypes
        bf = ml_dtypes.bfloat16
        wqT, wkT, wvT = wqT.astype(bf), wkT.astype(bf), wvT.astype(bf)
        pjT = pjT.astype(bf)

    g1 = np.zeros((C, GROUPS), np.float32)
    g1[np.arange(C), np.arange(C) // 4] = 0.25
    g2 = np.zeros((GROUPS, C), np.float32)
    g2[np.arange(C) // 4, np.arange(C)] = 1.0
    ebc = np.zeros((NH, C), np.float32)
    for h in range(NH):
        ebc[h, 32 * h:32 * h + 32] = 1.0

    params = dict(
        wqT=wqT, wkT=wkT, wvT=wvT, pjT=pjT,
        pjb=np.ascontiguousarray(proj_b[None, :]),
        nw=np.ascontiguousarray(norm_w[:, None]),
        nb=np.ascontiguousarray(norm_b[:, None]),
        g1=g1, g2=g2, ebc=ebc,
    )
    in_maps = []
    for i in range(NCORES):
        m = dict(params)
        m["x"] = np.ascontiguousarray(x[i * BPC:(i + 1) * BPC])
        in_maps.append(m)
    return in_maps


def kernel(**inputs):
    global LAST_RESULT
    from concourse.bass_utils import run_bass_kernel_spmd
    in_maps = _host_prep(inputs)
    nc = _get_nc()
    res = run_bass_kernel_spmd(nc, in_maps, list(range(NCORES)), trace=TRACE)
    LAST_RESULT = res
    y = np.concatenate([res.results[i]["y"] for i in range(NCORES)], axis=0)
    return y.reshape(B, C, 32, 32)


# revision 41
# speedup vs baseline: 1.1197x; 1.0237x over previous
"""AttentionBlock Trainium2 Bass kernel.

Full inputs -> shard batch over 8 NeuronCores (4 samples each) -> full output.

Per-sample on-core pipeline:
  x [C=128p, HW=1024f] -> groupnorm (bn_stats + PE group-reduce + affine)
  -> Q,K (per-head channel layout, scale^2 folded into Wq), V computed
     pre-transposed (Vt[s, (h,d)])
  -> S'[s,t] = K^T Q per head via K=32 matmuls row-tiled into PE quadrants;
     exp on ScalarE (no max subtraction: |S| < 1.3)
  -> hout via M=32 col-tiled matmuls (4 heads concurrent, standard channel
     order) + Z via M=1 ones-matmuls, accumulating over s-chunks in PSUM
  -> normalize with 1/Z (packed DVE reciprocal + PE broadcast)
  -> proj (+bias via K=1 ones matmul) + residual add -> y

float32r (single-pass fp32 matmul, bf16-rounded multiply) is used for the
qkv/attention/proj matmuls; groupnorm statistics stay exact fp32.
"""

import numpy as np
from contextlib import ExitStack

B, C, HW = 32, 128, 1024
NH, DH = 4, 32
GROUPS = 32
EPS = 1e-5
NCORES = 8
BPC = B // NCORES  # samples per core

_CACHE = {}
TRACE = False
LAST_RESULT = None
# matmul precision for qkv/attention/proj (groupnorm stats stay fp32):
#   "f32"  - exact, 4 cyc/col
#   "f32r" - stationary bf16-rounded, moving fp32, 2 cyc/col (~1.6e-4 rel)
#   "bf16" - both operands bf16, 1 cyc/col (~3e-3 rel; gate is 2e-2)
PREC = "bf16"
RLEVEL = 2  # legacy flag: host-side pre-round of weights for f32r


def _patch_tile_waits(tile, mybir):
    """This walrus build encodes only one sync-wait slot per instruction;
    Tile can attach several. Split extra waits onto NoOps committed
    immediately before the instruction on the same engine queue
    (in-order => identical semantics)."""
    if getattr(tile.TileContext, "_mm_wait_patched", False):
        return
    orig = tile.TileContext._commit_instruction

    def patched(self, inst, lazy_reg_writes=True):
        si = getattr(inst, "sync_info", None)
        if (not isinstance(inst, mybir.InstNoOp) and si is not None
                and si.on_wait and len(si.on_wait) > 1):
            waits = list(si.on_wait)
            for w in waits[:-1]:
                nop = mybir.InstNoOp(
                    name=self.nc.get_next_instruction_name(),
                    engine=inst.engine,
                    bass_nofuse=True,
                    sync_info=mybir.SyncInfo(on_wait=[w], on_update=[]),
                )
                orig(self, nop, lazy_reg_writes=False)
            inst.sync_info = mybir.SyncInfo(
                on_wait=[waits[-1]], on_update=list(si.on_update))
        return orig(self, inst, lazy_reg_writes)

    tile.TileContext._commit_instruction = patched

    def patched_drain(self, tick_clock, wait_clock):
        # Collect end-of-kernel waits, then hand them out one per SP nop
        # (the drain keeps none); nops precede the teardown barrier on the
        # same queue, so semantics are preserved.
        self.nc.sync.drain()
        sink = self.nc.sync.nop(nofuse=True)
        wait_clock.add_sem_waits(
            sink.ins, tile.ScopedClock({None: tick_clock.global_clock}))
        si = sink.ins.sync_info
        waits = list(si.on_wait) if si and si.on_wait else []
        if len(waits) > 1:
            sink.ins.sync_info = mybir.SyncInfo(
                on_wait=[waits[0]], on_update=list(si.on_update))
            for w in waits[1:]:
                extra = self.nc.sync.nop(nofuse=True)
                extra.ins.sync_info = mybir.SyncInfo(on_wait=[w], on_update=[])

        self.nc.all_engine_barrier()
        assert self.sems is not None
        popped = self.nc._tile_sem_poison_stack.pop()
        assert popped is self._sem_poison
        self.nc.clear_and_free_semaphores(list(self.sems.allocated().values()))
        self.nc.all_engine_barrier()

    tile.TileContext._drain_and_barrier = patched_drain
    tile.TileContext._mm_wait_patched = True


def _build_nc():
    import concourse.bass as bass
    import concourse.tile as tile
    from concourse import mybir

    _patch_tile_waits(tile, mybir)

    f32 = mybir.dt.float32
    _prec = {"f32": f32, "f32r": mybir.dt.float32r,
             "bf16": mybir.dt.bfloat16}[PREC]
    d1 = d2 = _prec
    nc = bass.Bass()

    x_d = nc.dram_tensor("x", [BPC, C, HW], f32, kind="ExternalInput")
    wq_d = nc.dram_tensor("wqT", [C, C], d2, kind="ExternalInput")
    wk_d = nc.dram_tensor("wkT", [C, C], d2, kind="ExternalInput")
    wv_d = nc.dram_tensor("wvT", [C, C], d2, kind="ExternalInput")
    pjT_d = nc.dram_tensor("pjT", [C, C], d1, kind="ExternalInput")
    pjb_d = nc.dram_tensor("pjb", [C, 1], f32, kind="ExternalInput")
    nw_d = nc.dram_tensor("nw", [C, 1], f32, kind="ExternalInput")
    nb_d = nc.dram_tensor("nb", [C, 1], f32, kind="ExternalInput")
    g1_d = nc.dram_tensor("g1", [C, GROUPS], f32, kind="ExternalInput")
    g2_d = nc.dram_tensor("g2", [GROUPS, C], f32, kind="ExternalInput")
    ebc_d = nc.dram_tensor("ebc", [NH, C], d1, kind="ExternalInput")
    y_d = nc.dram_tensor("y", [BPC, C, HW], f32, kind="ExternalOutput")

    with tile.TileContext(nc) as tc:
        with ExitStack() as ctx:
            _body(ctx, tc, mybir, bass, d1, d2,
                  x_d, wq_d, wk_d, wv_d, pjT_d, pjb_d, nw_d, nb_d,
                  g1_d, g2_d, ebc_d, y_d)
    return nc


def _body(ctx, tc, mybir, bass, d1, d2,
          x_d, wq_d, wk_d, wv_d, pjT_d, pjb_d, nw_d, nb_d, g1_d, g2_d,
          ebc_d, y_d):
    nc = tc.nc
    f32 = mybir.dt.float32
    AF = mybir.ActivationFunctionType
    OP = mybir.AluOpType

    const = ctx.enter_context(tc.tile_pool(name="const", bufs=1))
    sb_x = ctx.enter_context(tc.tile_pool(name="sb_x", bufs=2))
    sb_qk = ctx.enter_context(tc.tile_pool(name="sb_qk", bufs=2))
    sb_a = ctx.enter_context(tc.tile_pool(name="sb_a", bufs=3))
    sb_sm = ctx.enter_context(tc.tile_pool(name="sb_sm", bufs=4))
    # PSUM budget: ps_s 2x[128,1024] (4 banks) + ps_h tags h,z (4 banks) = 8
    ps_s = ctx.enter_context(tc.tile_pool(name="ps_s", bufs=2, space="PSUM"))
    ps_h = ctx.enter_context(tc.tile_pool(name="ps_h", bufs=1, space="PSUM"))

    # ---- constants ----
    wq_sb = const.tile([C, C], d2, tag="wq")
    wk_sb = const.tile([C, C], d2, tag="wk")
    wv_sb = const.tile([C, C], d2, tag="wv")
    pj_sb = const.tile([C, C], d1, tag="pj")
    pjb_sb = const.tile([C, 1], f32, tag="pjb")
    nw_sb = const.tile([C, 1], f32, tag="nw")
    nb_sb = const.tile([C, 1], f32, tag="nb")
    g1_sb = const.tile([C, GROUPS], f32, tag="g1")
    g2_sb = const.tile([GROUPS, C], f32, tag="g2")
    ebc_sb = const.tile([NH, C], d1, tag="ebc")
    for dst, src in ((wq_sb, wq_d), (wk_sb, wk_d), (wv_sb, wv_d),
                     (pj_sb, pjT_d), (pjb_sb, pjb_d), (nw_sb, nw_d),
                     (nb_sb, nb_d), (g1_sb, g1_d), (g2_sb, g2_d),
                     (ebc_sb, ebc_d)):
        nc.sync.dma_start(out=dst, in_=src[:])
    ones_sb = const.tile([1, HW], f32, tag="ones")
    nc.vector.memset(ones_sb, 1.0)
    eps_sb = const.tile([GROUPS, 1], f32, tag="eps")
    nc.vector.memset(eps_sb, EPS)
    ones32 = const.tile([C, DH], f32, tag="ones32")
    nc.vector.memset(ones32, 1.0)


    NHALF = HW // 2  # 512

    for b in range(BPC):
        # ---------- load ----------
        x_sb = sb_x.tile([C, HW], f32, tag="x")
        nc.sync.dma_start(out=x_sb, in_=x_d[b])

        # ---------- groupnorm ----------
        st6 = sb_sm.tile([C, 2, 6], f32, tag="st6")
        nc.vector.bn_stats(out=st6[:, 0, :], in_=x_sb[:, 0:512])
        nc.vector.bn_stats(out=st6[:, 1, :], in_=x_sb[:, 512:1024])
        mv = sb_sm.tile([C, 2], f32, tag="mv")
        nc.vector.bn_aggr(out=mv, in_=st6)
        # s2 = [mean_c, mean_c^2 + var_c]
        s2 = sb_sm.tile([C, 2], f32, tag="s2")
        nc.vector.tensor_copy(out=s2[:, 0:1], in_=mv[:, 0:1])
        nc.vector.tensor_mul(out=s2[:, 1:2], in0=mv[:, 0:1], in1=mv[:, 0:1])
        nc.vector.tensor_add(out=s2[:, 1:2], in0=s2[:, 1:2], in1=mv[:, 1:2])
        # group reduce: [32, 2] = (g1/4)^T @ s2
        gp = ps_s.tile([C, HW], f32, tag="s")
        nc.tensor.matmul(gp[0:GROUPS, 0:2], g1_sb, s2, start=True, stop=True)
        gs = sb_sm.tile([GROUPS, 2], f32, tag="gs")
        nc.vector.tensor_copy(out=gs, in_=gp[0:GROUPS, 0:2])
        # vv = [mu_g, rstd_g]; rstd = exp(-0.5*ln(var+eps)) (same ACT set)
        vv = sb_sm.tile([GROUPS, 2], f32, tag="vv")
        nc.vector.tensor_mul(out=vv[:, 0:1], in0=gs[:, 0:1], in1=gs[:, 0:1])
        nc.vector.tensor_tensor(out=vv[:, 1:2], in0=gs[:, 1:2], in1=vv[:, 0:1],
                                op=OP.subtract)
        nc.scalar.activation(out=vv[:, 1:2], in_=vv[:, 1:2], func=AF.Ln,
                             bias=eps_sb, scale=1.0)
        nc.scalar.activation(out=vv[:, 1:2], in_=vv[:, 1:2], func=AF.Exp,
                             bias=0.0, scale=-0.5)
        nc.vector.tensor_copy(out=vv[:, 0:1], in_=gs[:, 0:1])
        # broadcast to channels: bc[c, 0:2] = [mu_c, rstd_c]
        bc = ps_s.tile([C, HW], f32, tag="s")
        nc.tensor.matmul(bc[0:C, 0:2], g2_sb, vv, start=True, stop=True)
        aff = sb_sm.tile([C, 2], f32, tag="aff")
        nc.vector.tensor_mul(out=aff[:, 0:1], in0=nw_sb, in1=bc[:, 1:2])
        nc.vector.tensor_mul(out=aff[:, 1:2], in0=bc[:, 0:1], in1=aff[:, 0:1])
        nc.vector.tensor_tensor(out=aff[:, 1:2], in0=nb_sb, in1=aff[:, 1:2],
                                op=OP.subtract)
        xn = sb_x.tile([C, HW], d2, tag="xn")
        nc.vector.tensor_scalar(out=xn, in0=x_sb,
                                scalar1=aff[:, 0:1], scalar2=aff[:, 1:2],
                                op0=OP.mult, op1=OP.add)

        # ---------- Q, K, Vt ----------
        qp = ps_s.tile([C, HW], f32, tag="s")
        kp = ps_s.tile([C, HW], f32, tag="s")
        for n in range(2):
            sl = slice(n * NHALF, (n + 1) * NHALF)
            nc.tensor.matmul(qp[:, sl], wq_sb, xn[:, sl], start=True, stop=True)
            nc.tensor.matmul(kp[:, sl], wk_sb, xn[:, sl], start=True, stop=True)
        q_sb = sb_qk.tile([C, HW], d1, tag="q")
        k_sb = sb_qk.tile([C, HW], d1, tag="k")
        nc.vector.tensor_copy(out=q_sb, in_=qp)
        nc.vector.tensor_copy(out=k_sb, in_=kp)

        vp = ps_s.tile([C, HW], f32, tag="s")
        for j in range(8):
            nc.tensor.matmul(vp[:, j * 128:(j + 1) * 128],
                             xn[:, j * 128:(j + 1) * 128], wv_sb,
                             start=True, stop=True)
        vt = sb_qk.tile([C, 8, NH, 33], d1, tag="vt")
        nc.vector.tensor_copy(
            out=vt[:, :, :, 32:33],
            in_=ones32.rearrange("p (a b c) -> p a b c", a=8, b=NH))
        for j in range(8):
            src_v = vp[:, j * 128:(j + 1) * 128].rearrange(
                "p (h d) -> p h d", d=DH)
            nc.vector.tensor_copy(out=vt[:, j, :, 0:DH], in_=src_v)

        # ---------- attention (head pairs; SV = M=33 at base 0) ----------
        hun = sb_x.tile([C, HW], f32, tag="hun")
        zp = sb_sm.tile([C, NH, 8], f32, tag="zp")
        for pair in range(2):
            ha0 = ps_h.tile([33, HW], f32, tag="ha0")
            ha1 = ps_h.tile([33, HW], f32, tag="ha1")
            ha = {0: ha0, 1: ha1}
            for j in range(8):
                a_sb = sb_a.tile([C, 2, HW], d1, tag="a")
                sp0 = ps_s.tile([C, HW], f32, tag="s")
                sp1 = ps_s.tile([C, HW], f32, tag="s")
                sps = (sp0, sp1)
                # S' for both heads first, head-interleaved so the two row
                # groups stream concurrently in the PE array
                for n in range(2):
                    sl = slice(n * NHALF, (n + 1) * NHALF)
                    for k in range(2):
                        h = 2 * pair + k
                        hp = slice(32 * h, 32 * h + 32)
                        nc.tensor.matmul(
                            sps[k][:, sl],
                            k_sb[hp, j * 128:(j + 1) * 128],
                            q_sb[hp, sl],
                            start=True, stop=True,
                            tile_position=(32 * h, 0))
                for k in range(2):
                    nc.scalar.activation(out=a_sb[:, k, :], in_=sps[k],
                                         func=AF.Exp, bias=0.0, scale=1.0)
                for k in range(2):
                    h = 2 * pair + k
                    for n in range(2):
                        sl = slice(n * NHALF, (n + 1) * NHALF)
                        nc.tensor.matmul(
                            ha[k][:, sl],
                            vt[:, j, h, :],
                            a_sb[:, k, sl],
                            start=(j == 0), stop=(j == 7))
            for k in range(2):
                h = 2 * pair + k
                hv = sb_sm.tile([33, HW], f32, tag="hv%d" % k)
                nc.vector.tensor_copy(out=hv, in_=ha[k])
                nc.sync.dma_start(out=zp[:, h, :], in_=hv[32:33, :])
                nc.sync.dma_start(out=hun[32 * h:32 * h + 32, :],
                                  in_=hv[0:32, :])

        # ----- 1/Z and normalize -----
        rp = sb_sm.tile([C, NH, 8], d1, tag="rp")
        with nc.allow_low_precision(reason="1/Z broadcast in bf16; gate 2e-2"):
            nc.vector.reciprocal(out=rp, in_=zp)
        ral = sb_sm.tile([NH, HW], d1, tag="ral")
        for h in range(NH):
            nc.sync.dma_start(out=ral[h:h + 1, :], in_=rp[:, h, :])
        rb_ps = ps_s.tile([C, HW], f32, tag="s")
        for n in range(2):
            sl = slice(n * NHALF, (n + 1) * NHALF)
            nc.tensor.matmul(rb_ps[:, sl], ebc_sb, ral[:, sl],
                             start=True, stop=True)
        rb = sb_x.tile([C, HW], f32, tag="rb")
        nc.vector.tensor_copy(out=rb, in_=rb_ps)
        hn = sb_x.tile([C, HW], d1, tag="hn")
        nc.vector.tensor_mul(out=hn, in0=hun, in1=rb)

        # ---------- proj + bias + residual ----------
        pp = ps_s.tile([C, HW], f32, tag="s")
        for n in range(2):
            sl = slice(n * NHALF, (n + 1) * NHALF)
            nc.tensor.matmul(pp[:, sl], pj_sb, hn[:, sl],
                             start=True, stop=True)
        out_sb = sb_x.tile([C, HW], f32, tag="out")
        nc.vector.tensor_scalar(out=out_sb, in0=pp, scalar1=pjb_sb,
                                scalar2=None, op0=OP.add)
        nc.vector.tensor_add(out=out_sb, in0=out_sb, in1=x_sb)
        nc.sync.dma_start(out=y_d[b], in_=out_sb)


def _get_nc():
    if "nc" not in _CACHE:
        _CACHE["nc"] = _build_nc()
    return _CACHE["nc"]


def _bf16_round(a):
    b = np.ascontiguousarray(a, np.float32).view(np.uint32)
    b = (b + 0x8000 - ((b >> 16) & 1)) & 0xFFFF0000
    return b.view(np.float32)


def _host_prep(inputs):
    x = np.ascontiguousarray(
        np.asarray(inputs["x"], np.float32).reshape(B, C, HW))
    qkv_w = np.asarray(inputs["qkv_w"], np.float32)
    proj_w = np.asarray(inputs["proj_w"], np.float32)
    proj_b = np.asarray(inputs["proj_b"], np.float32)
    norm_w = np.asarray(inputs["norm_w"], np.float32)
    norm_b = np.asarray(inputs["norm_b"], np.float32)

    w3 = qkv_w.reshape(NH, 3, DH, C)  # rows: h*96 + which*32 + d
    wq = w3[:, 0].reshape(C, C)
    wk = w3[:, 1].reshape(C, C)
    wv = w3[:, 2].reshape(C, C)
    wqT = np.ascontiguousarray((wq / 32.0).T)  # fold scale^2 = 1/dh
    wkT = np.ascontiguousarray(wk.T)
    wvT = np.ascontiguousarray(wv.T)
    pjT = np.ascontiguousarray(proj_w.T)
    if PREC == "f32r":
        wqT, wkT, wvT = _bf16_round(wqT), _bf16_round(wkT), _bf16_round(wvT)
        pjT = _bf16_round(pjT)
    elif PREC == "bf16":
        import ml_dtypes
        bf = ml_dtypes.bfloat16
        wqT, wkT, wvT = wqT.astype(bf), wkT.astype(bf), wvT.astype(bf)
        pjT = pjT.astype(bf)

    g1 = np.zeros((C, GROUPS), np.float32)
    g1[np.arange(C), np.arange(C) // 4] = 0.25
    g2 = np.zeros((GROUPS, C), np.float32)
    g2[np.arange(C) // 4, np.arange(C)] = 1.0
    ebc = np.zeros((NH, C), np.float32)
    for h in range(NH):
        ebc[h, 32 * h:32 * h + 32] = 1.0
    if PREC == "bf16":
        import ml_dtypes
        ebc = ebc.astype(ml_dtypes.bfloat16)

    params = dict(
        wqT=wqT, wkT=wkT, wvT=wvT, pjT=pjT,
        pjb=np.ascontiguousarray(proj_b[:, None]),
        nw=np.ascontiguousarray(norm_w[:, None]),
        nb=np.ascontiguousarray(norm_b[:, None]),
        g1=g1, g2=g2, ebc=ebc,
    )
    in_maps = []
    for i in range(NCORES):
        m = dict(params)
        m["x"] = np.ascontiguousarray(x[i * BPC:(i + 1) * BPC])
        in_maps.append(m)
    return in_maps


def kernel(**inputs):
    global LAST_RESULT
    from concourse.bass_utils import run_bass_kernel_spmd
    in_maps = _host_prep(inputs)
    nc = _get_nc()
    res = run_bass_kernel_spmd(nc, in_maps, list(range(NCORES)), trace=TRACE)
    LAST_RESULT = res
    y = np.concatenate([res.results[i]["y"] for i in range(NCORES)], axis=0)
    return y.reshape(B, C, 32, 32)


# revision 42
# speedup vs baseline: 1.2591x; 1.1245x over previous
"""AttentionBlock Trainium2 Bass kernel.

Full inputs -> shard batch over 8 NeuronCores (4 samples each) -> full output.

Per-sample on-core pipeline:
  x [C=128p, HW=1024f] -> groupnorm (bn_stats + PE group-reduce + affine)
  -> Q,K (per-head channel layout, scale^2 folded into Wq), V computed
     pre-transposed (Vt[s, (h,d)])
  -> S'[s,t] = K^T Q per head via K=32 matmuls row-tiled into PE quadrants;
     exp on ScalarE (no max subtraction: |S| < 1.3)
  -> hout via M=32 col-tiled matmuls (4 heads concurrent, standard channel
     order) + Z via M=1 ones-matmuls, accumulating over s-chunks in PSUM
  -> normalize with 1/Z (packed DVE reciprocal + PE broadcast)
  -> proj (+bias via K=1 ones matmul) + residual add -> y

float32r (single-pass fp32 matmul, bf16-rounded multiply) is used for the
qkv/attention/proj matmuls; groupnorm statistics stay exact fp32.
"""

import numpy as np
from contextlib import ExitStack

B, C, HW = 32, 128, 1024
NH, DH = 4, 32
GROUPS = 32
EPS = 1e-5
NCORES = 8
BPC = B // NCORES  # samples per core

_CACHE = {}
TRACE = False
LAST_RESULT = None
# matmul precision for qkv/attention/proj (groupnorm stats stay fp32):
#   "f32"  - exact, 4 cyc/col
#   "f32r" - stationary bf16-rounded, moving fp32, 2 cyc/col (~1.6e-4 rel)
#   "bf16" - both operands bf16, 1 cyc/col (~3e-3 rel; gate is 2e-2)
PREC = "bf16"
RLEVEL = 2  # legacy flag: host-side pre-round of weights for f32r


def _patch_tile_waits(tile, mybir):
    """This walrus build encodes only one sync-wait slot per instruction;
    Tile can attach several. Split extra waits onto NoOps committed
    immediately before the instruction on the same engine queue
    (in-order => identical semantics)."""
    if getattr(tile.TileContext, "_mm_wait_patched", False):
        return
    orig = tile.TileContext._commit_instruction

    def patched(self, inst, lazy_reg_writes=True):
        si = getattr(inst, "sync_info", None)
        if (not isinstance(inst, mybir.InstNoOp) and si is not None
                and si.on_wait and len(si.on_wait) > 1):
            waits = list(si.on_wait)
            for w in waits[:-1]:
                nop = mybir.InstNoOp(
                    name=self.nc.get_next_instruction_name(),
                    engine=inst.engine,
                    bass_nofuse=True,
                    sync_info=mybir.SyncInfo(on_wait=[w], on_update=[]),
                )
                orig(self, nop, lazy_reg_writes=False)
            inst.sync_info = mybir.SyncInfo(
                on_wait=[waits[-1]], on_update=list(si.on_update))
        return orig(self, inst, lazy_reg_writes)

    tile.TileContext._commit_instruction = patched

    def patched_drain(self, tick_clock, wait_clock):
        # Collect end-of-kernel waits, then hand them out one per SP nop
        # (the drain keeps none); nops precede the teardown barrier on the
        # same queue, so semantics are preserved.
        self.nc.sync.drain()
        sink = self.nc.sync.nop(nofuse=True)
        wait_clock.add_sem_waits(
            sink.ins, tile.ScopedClock({None: tick_clock.global_clock}))
        si = sink.ins.sync_info
        waits = list(si.on_wait) if si and si.on_wait else []
        if len(waits) > 1:
            sink.ins.sync_info = mybir.SyncInfo(
                on_wait=[waits[0]], on_update=list(si.on_update))
            for w in waits[1:]:
                extra = self.nc.sync.nop(nofuse=True)
                extra.ins.sync_info = mybir.SyncInfo(on_wait=[w], on_update=[])

        self.nc.all_engine_barrier()
        assert self.sems is not None
        popped = self.nc._tile_sem_poison_stack.pop()
        assert popped is self._sem_poison
        self.nc.clear_and_free_semaphores(list(self.sems.allocated().values()))
        self.nc.all_engine_barrier()

    tile.TileContext._drain_and_barrier = patched_drain
    tile.TileContext._mm_wait_patched = True


def _build_nc():
    import concourse.bass as bass
    import concourse.tile as tile
    from concourse import mybir

    _patch_tile_waits(tile, mybir)

    f32 = mybir.dt.float32
    _prec = {"f32": f32, "f32r": mybir.dt.float32r,
             "bf16": mybir.dt.bfloat16}[PREC]
    d1 = d2 = _prec
    nc = bass.Bass()

    x_d = nc.dram_tensor("x", [BPC, C, HW], f32, kind="ExternalInput")
    wq_d = nc.dram_tensor("wqT", [C, C], d2, kind="ExternalInput")
    wk_d = nc.dram_tensor("wkT", [C, C], d2, kind="ExternalInput")
    wv_d = nc.dram_tensor("wvT", [C, C], d2, kind="ExternalInput")
    pjT_d = nc.dram_tensor("pjT", [C, C], d1, kind="ExternalInput")
    pjb_d = nc.dram_tensor("pjb", [C, 1], f32, kind="ExternalInput")
    nw_d = nc.dram_tensor("nw", [C, 1], f32, kind="ExternalInput")
    nb_d = nc.dram_tensor("nb", [C, 1], f32, kind="ExternalInput")
    g1_d = nc.dram_tensor("g1", [C, GROUPS], f32, kind="ExternalInput")
    g2_d = nc.dram_tensor("g2", [GROUPS, C], f32, kind="ExternalInput")
    ebc_d = nc.dram_tensor("ebc", [NH, C], d1, kind="ExternalInput")
    y_d = nc.dram_tensor("y", [BPC, C, HW], f32, kind="ExternalOutput")

    with tile.TileContext(nc) as tc:
        with ExitStack() as ctx:
            _body(ctx, tc, mybir, bass, d1, d2,
                  x_d, wq_d, wk_d, wv_d, pjT_d, pjb_d, nw_d, nb_d,
                  g1_d, g2_d, ebc_d, y_d)
    return nc


def _body(ctx, tc, mybir, bass, d1, d2,
          x_d, wq_d, wk_d, wv_d, pjT_d, pjb_d, nw_d, nb_d, g1_d, g2_d,
          ebc_d, y_d):
    nc = tc.nc
    f32 = mybir.dt.float32
    AF = mybir.ActivationFunctionType
    OP = mybir.AluOpType

    const = ctx.enter_context(tc.tile_pool(name="const", bufs=1))
    sb_x = ctx.enter_context(tc.tile_pool(name="sb_x", bufs=3))
    sb_qk = ctx.enter_context(tc.tile_pool(name="sb_qk", bufs=3))
    sb_a = ctx.enter_context(tc.tile_pool(name="sb_a", bufs=4))
    sb_sm = ctx.enter_context(tc.tile_pool(name="sb_sm", bufs=4))
    # PSUM budget: ps_s 2x[128,1024] (4 banks) + ps_h tags h,z (4 banks) = 8
    ps_s = ctx.enter_context(tc.tile_pool(name="ps_s", bufs=2, space="PSUM"))
    ps_h = ctx.enter_context(tc.tile_pool(name="ps_h", bufs=1, space="PSUM"))

    # ---- constants ----
    wq_sb = const.tile([C, C], d2, tag="wq")
    wk_sb = const.tile([C, C], d2, tag="wk")
    wv_sb = const.tile([C, C], d2, tag="wv")
    pj_sb = const.tile([C, C], d1, tag="pj")
    pjb_sb = const.tile([C, 1], f32, tag="pjb")
    nw_sb = const.tile([C, 1], f32, tag="nw")
    nb_sb = const.tile([C, 1], f32, tag="nb")
    g1_sb = const.tile([C, GROUPS], f32, tag="g1")
    g2_sb = const.tile([GROUPS, C], f32, tag="g2")
    ebc_sb = const.tile([NH, C], d1, tag="ebc")
    for dst, src in ((wq_sb, wq_d), (wk_sb, wk_d), (wv_sb, wv_d),
                     (pj_sb, pjT_d), (pjb_sb, pjb_d), (nw_sb, nw_d),
                     (nb_sb, nb_d), (g1_sb, g1_d), (g2_sb, g2_d),
                     (ebc_sb, ebc_d)):
        nc.sync.dma_start(out=dst, in_=src[:])
    ones_sb = const.tile([1, HW], f32, tag="ones")
    nc.vector.memset(ones_sb, 1.0)
    eps_sb = const.tile([GROUPS, 1], f32, tag="eps")
    nc.vector.memset(eps_sb, EPS)
    ones32 = const.tile([C, DH], f32, tag="ones32")
    nc.vector.memset(ones32, 1.0)


    NHALF = HW // 2  # 512

    for b in range(BPC):
        # ---------- load ----------
        x_sb = sb_x.tile([C, HW], f32, tag="x")
        nc.sync.dma_start(out=x_sb, in_=x_d[b])
        xb2 = sb_x.tile([C, HW], f32, tag="xb2")
        nc.vector.tensor_scalar(out=xb2, in0=x_sb, scalar1=pjb_sb,
                                scalar2=None, op0=OP.add)

        # ---------- groupnorm ----------
        st6 = sb_sm.tile([C, 2, 6], f32, tag="st6")
        nc.vector.bn_stats(out=st6[:, 0, :], in_=x_sb[:, 0:512])
        nc.vector.bn_stats(out=st6[:, 1, :], in_=x_sb[:, 512:1024])
        mv = sb_sm.tile([C, 2], f32, tag="mv")
        nc.vector.bn_aggr(out=mv, in_=st6)
        # s2 = [mean_c, mean_c^2 + var_c]
        s2 = sb_sm.tile([C, 2], f32, tag="s2")
        nc.vector.tensor_copy(out=s2[:, 0:1], in_=mv[:, 0:1])
        nc.vector.tensor_mul(out=s2[:, 1:2], in0=mv[:, 0:1], in1=mv[:, 0:1])
        nc.vector.tensor_add(out=s2[:, 1:2], in0=s2[:, 1:2], in1=mv[:, 1:2])
        # group reduce: [32, 2] = (g1/4)^T @ s2
        gp = ps_s.tile([C, HW], f32, tag="s")
        nc.tensor.matmul(gp[0:GROUPS, 0:2], g1_sb, s2, start=True, stop=True)
        gs = sb_sm.tile([GROUPS, 2], f32, tag="gs")
        nc.vector.tensor_copy(out=gs, in_=gp[0:GROUPS, 0:2])
        # vv = [mu_g, rstd_g]; rstd = exp(-0.5*ln(var+eps)) (same ACT set)
        vv = sb_sm.tile([GROUPS, 2], f32, tag="vv")
        nc.vector.tensor_mul(out=vv[:, 0:1], in0=gs[:, 0:1], in1=gs[:, 0:1])
        nc.vector.tensor_tensor(out=vv[:, 1:2], in0=gs[:, 1:2], in1=vv[:, 0:1],
                                op=OP.subtract)
        nc.scalar.activation(out=vv[:, 1:2], in_=vv[:, 1:2], func=AF.Ln,
                             bias=eps_sb, scale=1.0)
        nc.scalar.activation(out=vv[:, 1:2], in_=vv[:, 1:2], func=AF.Exp,
                             bias=0.0, scale=-0.5)
        nc.vector.tensor_copy(out=vv[:, 0:1], in_=gs[:, 0:1])
        # broadcast to channels: bc[c, 0:2] = [mu_c, rstd_c]
        bc = ps_s.tile([C, HW], f32, tag="s")
        nc.tensor.matmul(bc[0:C, 0:2], g2_sb, vv, start=True, stop=True)
        aff = sb_sm.tile([C, 2], f32, tag="aff")
        nc.vector.tensor_mul(out=aff[:, 0:1], in0=nw_sb, in1=bc[:, 1:2])
        nc.vector.tensor_mul(out=aff[:, 1:2], in0=bc[:, 0:1], in1=aff[:, 0:1])
        nc.vector.tensor_tensor(out=aff[:, 1:2], in0=nb_sb, in1=aff[:, 1:2],
                                op=OP.subtract)
        xn = sb_x.tile([C, HW], d2, tag="xn")
        nc.vector.tensor_scalar(out=xn, in0=x_sb,
                                scalar1=aff[:, 0:1], scalar2=aff[:, 1:2],
                                op0=OP.mult, op1=OP.add)

        # ---------- Q, K, Vt ----------
        qp = ps_s.tile([C, HW], f32, tag="s")
        kp = ps_s.tile([C, HW], f32, tag="s")
        for n in range(2):
            sl = slice(n * NHALF, (n + 1) * NHALF)
            nc.tensor.matmul(qp[:, sl], wq_sb, xn[:, sl], start=True, stop=True)
            nc.tensor.matmul(kp[:, sl], wk_sb, xn[:, sl], start=True, stop=True)
        q_sb = sb_qk.tile([C, HW], d1, tag="q")
        k_sb = sb_qk.tile([C, HW], d1, tag="k")
        nc.vector.tensor_copy(out=q_sb, in_=qp)
        nc.vector.tensor_copy(out=k_sb, in_=kp)

        vp = ps_s.tile([C, HW], f32, tag="s")
        for j in range(8):
            nc.tensor.matmul(vp[:, j * 128:(j + 1) * 128],
                             xn[:, j * 128:(j + 1) * 128], wv_sb,
                             start=True, stop=True)
        vt = sb_qk.tile([C, 8, NH, 33], d1, tag="vt")
        nc.vector.tensor_copy(
            out=vt[:, :, :, 32:33],
            in_=ones32.rearrange("p (a b c) -> p a b c", a=8, b=NH))
        for j in range(8):
            src_v = vp[:, j * 128:(j + 1) * 128].rearrange(
                "p (h d) -> p h d", d=DH)
            nc.vector.tensor_copy(out=vt[:, j, :, 0:DH], in_=src_v)

        # ---------- attention (head pairs; SV = M=33 at base 0) ----------
        hun = sb_x.tile([C, HW], f32, tag="hun")
        zp = sb_sm.tile([C, NH, 8], f32, tag="zp")
        for pair in range(2):
            ha0 = ps_h.tile([33, HW], f32, tag="ha0")
            ha1 = ps_h.tile([33, HW], f32, tag="ha1")
            ha = {0: ha0, 1: ha1}
            for j in range(8):
                a_sb = sb_a.tile([C, 2, HW], d1, tag="a")
                sp0 = ps_s.tile([C, HW], f32, tag="s")
                sp1 = ps_s.tile([C, HW], f32, tag="s")
                sps = (sp0, sp1)
                # S' for both heads first, head-interleaved so the two row
                # groups stream concurrently in the PE array
                for n in range(2):
                    sl = slice(n * NHALF, (n + 1) * NHALF)
                    for k in range(2):
                        h = 2 * pair + k
                        hp = slice(32 * h, 32 * h + 32)
                        nc.tensor.matmul(
                            sps[k][:, sl],
                            k_sb[hp, j * 128:(j + 1) * 128],
                            q_sb[hp, sl],
                            start=True, stop=True,
                            tile_position=(32 * h, 0))
                for k in range(2):
                    nc.scalar.activation(out=a_sb[:, k, :], in_=sps[k],
                                         func=AF.Exp, bias=0.0, scale=1.0)
                for k in range(2):
                    h = 2 * pair + k
                    for n in range(2):
                        sl = slice(n * NHALF, (n + 1) * NHALF)
                        nc.tensor.matmul(
                            ha[k][:, sl],
                            vt[:, j, h, :],
                            a_sb[:, k, sl],
                            start=(j == 0), stop=(j == 7))
            for k in range(2):
                h = 2 * pair + k
                hv = sb_sm.tile([33, HW], f32, tag="hv%d" % k)
                nc.vector.tensor_copy(out=hv, in_=ha[k])
                nc.sync.dma_start(out=zp[:, h, :], in_=hv[32:33, :])
                nc.sync.dma_start(out=hun[32 * h:32 * h + 32, :],
                                  in_=hv[0:32, :])

        # ----- 1/Z and normalize -----
        rp = sb_sm.tile([C, NH, 8], d1, tag="rp")
        with nc.allow_low_precision(reason="1/Z broadcast in bf16; gate 2e-2"):
            nc.vector.reciprocal(out=rp, in_=zp)
        ral = sb_sm.tile([NH, HW], d1, tag="ral")
        for h in range(NH):
            nc.sync.dma_start(out=ral[h:h + 1, :], in_=rp[:, h, :])
        rb_ps = ps_h.tile([C, HW], f32, tag="ha0")
        for n in range(2):
            sl = slice(n * NHALF, (n + 1) * NHALF)
            nc.tensor.matmul(rb_ps[:, sl], ebc_sb, ral[:, sl],
                             start=True, stop=True)
        hn = sb_x.tile([C, HW], d1, tag="hn")
        nc.vector.tensor_mul(out=hn, in0=hun, in1=rb_ps)

        # ---------- proj + bias + residual ----------
        pp = ps_h.tile([C, HW], f32, tag="ha1")
        for n in range(2):
            sl = slice(n * NHALF, (n + 1) * NHALF)
            nc.tensor.matmul(pp[:, sl], pj_sb, hn[:, sl],
                             start=True, stop=True)
        out_sb = sb_x.tile([C, HW], f32, tag="out")
        nc.vector.tensor_add(out=out_sb, in0=pp, in1=xb2)
        nc.sync.dma_start(out=y_d[b], in_=out_sb)


def _get_nc():
    if "nc" not in _CACHE:
        _CACHE["nc"] = _build_nc()
    return _CACHE["nc"]


def _bf16_round(a):
    b = np.ascontiguousarray(a, np.float32).view(np.uint32)
    b = (b + 0x8000 - ((b >> 16) & 1)) & 0xFFFF0000
    return b.view(np.float32)


def _host_prep(inputs):
    x = np.ascontiguousarray(
        np.asarray(inputs["x"], np.float32).reshape(B, C, HW))
    qkv_w = np.asarray(inputs["qkv_w"], np.float32)
    proj_w = np.asarray(inputs["proj_w"], np.float32)
    proj_b = np.asarray(inputs["proj_b"], np.float32)
    norm_w = np.asarray(inputs["norm_w"], np.float32)
    norm_b = np.asarray(inputs["norm_b"], np.float32)

    w3 = qkv_w.reshape(NH, 3, DH, C)  # rows: h*96 + which*32 + d
    wq = w3[:, 0].reshape(C, C)
    wk = w3[:, 1].reshape(C, C)
    wv = w3[:, 2].reshape(C, C)
    wqT = np.ascontiguousarray((wq / 32.0).T)  # fold scale^2 = 1/dh
    wkT = np.ascontiguousarray(wk.T)
    wvT = np.ascontiguousarray(wv.T)
    pjT = np.ascontiguousarray(proj_w.T)
    if PREC == "f32r":
        wqT, wkT, wvT = _bf16_round(wqT), _bf16_round(wkT), _bf16_round(wvT)
        pjT = _bf16_round(pjT)
    elif PREC == "bf16":
        import ml_dtypes
        bf = ml_dtypes.bfloat16
        wqT, wkT, wvT = wqT.astype(bf), wkT.astype(bf), wvT.astype(bf)
        pjT = pjT.astype(bf)

    g1 = np.zeros((C, GROUPS), np.float32)
    g1[np.arange(C), np.arange(C) // 4] = 0.25
    g2 = np.zeros((GROUPS, C), np.float32)
    g2[np.arange(C) // 4, np.arange(C)] = 1.0
    ebc = np.zeros((NH, C), np.float32)
    for h in range(NH):
        ebc[h, 32 * h:32 * h + 32] = 1.0
    if PREC == "bf16":
        import ml_dtypes
        ebc = ebc.astype(ml_dtypes.bfloat16)

    params = dict(
        wqT=wqT, wkT=wkT, wvT=wvT, pjT=pjT,
        pjb=np.ascontiguousarray(proj_b[:, None]),
        nw=np.ascontiguousarray(norm_w[:, None]),
        nb=np.ascontiguousarray(norm_b[:, None]),
        g1=g1, g2=g2, ebc=ebc,
    )
    in_maps = []
    for i in range(NCORES):
        m = dict(params)
        m["x"] = np.ascontiguousarray(x[i * BPC:(i + 1) * BPC])
        in_maps.append(m)
    return in_maps


def kernel(**inputs):
    global LAST_RESULT
    from concourse.bass_utils import run_bass_kernel_spmd
    in_maps = _host_prep(inputs)
    nc = _get_nc()
    res = run_bass_kernel_spmd(nc, in_maps, list(range(NCORES)), trace=TRACE)
    LAST_RESULT = res
    y = np.concatenate([res.results[i]["y"] for i in range(NCORES)], axis=0)
    return y.reshape(B, C, 32, 32)


# revision 44
# speedup vs baseline: 1.4940x; 1.1866x over previous
"""AttentionBlock Trainium2 Bass kernel.

Full inputs -> shard batch over 8 NeuronCores (4 samples each) -> full output.

Per-sample on-core pipeline:
  x [C=128p, HW=1024f] -> groupnorm (bn_stats + PE group-reduce + affine)
  -> Q,K (per-head channel layout, scale^2 folded into Wq), V computed
     pre-transposed (Vt[s, (h,d)])
  -> S'[s,t] = K^T Q per head via K=32 matmuls row-tiled into PE quadrants;
     exp on ScalarE (no max subtraction: |S| < 1.3)
  -> hout via M=32 col-tiled matmuls (4 heads concurrent, standard channel
     order) + Z via M=1 ones-matmuls, accumulating over s-chunks in PSUM
  -> normalize with 1/Z (packed DVE reciprocal + PE broadcast)
  -> proj (+bias via K=1 ones matmul) + residual add -> y

float32r (single-pass fp32 matmul, bf16-rounded multiply) is used for the
qkv/attention/proj matmuls; groupnorm statistics stay exact fp32.
"""

import numpy as np
from contextlib import ExitStack

B, C, HW = 32, 128, 1024
NH, DH = 4, 32
GROUPS = 32
EPS = 1e-5
NCORES = 8
BPC = B // NCORES  # samples per core

_CACHE = {}
TRACE = False
LAST_RESULT = None
# matmul precision for qkv/attention/proj (groupnorm stats stay fp32):
#   "f32"  - exact, 4 cyc/col
#   "f32r" - stationary bf16-rounded, moving fp32, 2 cyc/col (~1.6e-4 rel)
#   "bf16" - both operands bf16, 1 cyc/col (~3e-3 rel; gate is 2e-2)
PREC = "bf16"
RLEVEL = 2  # legacy flag: host-side pre-round of weights for f32r


def _patch_tile_waits(tile, mybir):
    """This walrus build encodes only one sync-wait slot per instruction;
    Tile can attach several. Split extra waits onto NoOps committed
    immediately before the instruction on the same engine queue
    (in-order => identical semantics)."""
    if getattr(tile.TileContext, "_mm_wait_patched", False):
        return
    orig = tile.TileContext._commit_instruction

    def patched(self, inst, lazy_reg_writes=True):
        si = getattr(inst, "sync_info", None)
        if (not isinstance(inst, mybir.InstNoOp) and si is not None
                and si.on_wait and len(si.on_wait) > 1):
            waits = list(si.on_wait)
            for w in waits[:-1]:
                nop = mybir.InstNoOp(
                    name=self.nc.get_next_instruction_name(),
                    engine=inst.engine,
                    bass_nofuse=True,
                    sync_info=mybir.SyncInfo(on_wait=[w], on_update=[]),
                )
                orig(self, nop, lazy_reg_writes=False)
            inst.sync_info = mybir.SyncInfo(
                on_wait=[waits[-1]], on_update=list(si.on_update))
        return orig(self, inst, lazy_reg_writes)

    tile.TileContext._commit_instruction = patched

    def patched_drain(self, tick_clock, wait_clock):
        # Collect end-of-kernel waits, then hand them out one per SP nop
        # (the drain keeps none); nops precede the teardown barrier on the
        # same queue, so semantics are preserved.
        self.nc.sync.drain()
        sink = self.nc.sync.nop(nofuse=True)
        wait_clock.add_sem_waits(
            sink.ins, tile.ScopedClock({None: tick_clock.global_clock}))
        si = sink.ins.sync_info
        waits = list(si.on_wait) if si and si.on_wait else []
        if len(waits) > 1:
            sink.ins.sync_info = mybir.SyncInfo(
                on_wait=[waits[0]], on_update=list(si.on_update))
            for w in waits[1:]:
                extra = self.nc.sync.nop(nofuse=True)
                extra.ins.sync_info = mybir.SyncInfo(on_wait=[w], on_update=[])

        self.nc.all_engine_barrier()
        assert self.sems is not None
        popped = self.nc._tile_sem_poison_stack.pop()
        assert popped is self._sem_poison
        self.nc.clear_and_free_semaphores(list(self.sems.allocated().values()))
        self.nc.all_engine_barrier()

    tile.TileContext._drain_and_barrier = patched_drain
    tile.TileContext._mm_wait_patched = True


def _build_nc():
    import concourse.bass as bass
    import concourse.tile as tile
    from concourse import mybir

    _patch_tile_waits(tile, mybir)

    f32 = mybir.dt.float32
    _prec = {"f32": f32, "f32r": mybir.dt.float32r,
             "bf16": mybir.dt.bfloat16}[PREC]
    d1 = d2 = _prec
    nc = bass.Bass()

    x_d = nc.dram_tensor("x", [BPC, C, HW], f32, kind="ExternalInput")
    wq_d = nc.dram_tensor("wqT", [C, C], d2, kind="ExternalInput")
    wk_d = nc.dram_tensor("wkT", [C, C], d2, kind="ExternalInput")
    wv_d = nc.dram_tensor("wvT", [C, C], d2, kind="ExternalInput")
    pjT_d = nc.dram_tensor("pjT", [C, C], d1, kind="ExternalInput")
    pjb_d = nc.dram_tensor("pjb", [C, 1], f32, kind="ExternalInput")
    nw_d = nc.dram_tensor("nw", [C, 1], f32, kind="ExternalInput")
    nb_d = nc.dram_tensor("nb", [C, 1], f32, kind="ExternalInput")
    g1_d = nc.dram_tensor("g1", [C, GROUPS], f32, kind="ExternalInput")
    g2_d = nc.dram_tensor("g2", [GROUPS, C], f32, kind="ExternalInput")
    ebc_d = nc.dram_tensor("ebc", [NH, C], d1, kind="ExternalInput")
    y_d = nc.dram_tensor("y", [BPC, C, HW], f32, kind="ExternalOutput")

    with tile.TileContext(nc) as tc:
        with ExitStack() as ctx:
            _body(ctx, tc, mybir, bass, d1, d2,
                  x_d, wq_d, wk_d, wv_d, pjT_d, pjb_d, nw_d, nb_d,
                  g1_d, g2_d, ebc_d, y_d)
    return nc


def _body(ctx, tc, mybir, bass, d1, d2,
          x_d, wq_d, wk_d, wv_d, pjT_d, pjb_d, nw_d, nb_d, g1_d, g2_d,
          ebc_d, y_d):
    nc = tc.nc
    f32 = mybir.dt.float32
    AF = mybir.ActivationFunctionType
    OP = mybir.AluOpType

    const = ctx.enter_context(tc.tile_pool(name="const", bufs=1))
    sb_x = ctx.enter_context(tc.tile_pool(name="sb_x", bufs=3))
    sb_qk = ctx.enter_context(tc.tile_pool(name="sb_qk", bufs=3))
    sb_a = ctx.enter_context(tc.tile_pool(name="sb_a", bufs=4))
    sb_sm = ctx.enter_context(tc.tile_pool(name="sb_sm", bufs=4))
    # PSUM budget: ps_s 2x[128,1024] (4 banks) + ps_h tags h,z (4 banks) = 8
    ps_s = ctx.enter_context(tc.tile_pool(name="ps_s", bufs=2, space="PSUM"))
    ps_h = ctx.enter_context(tc.tile_pool(name="ps_h", bufs=1, space="PSUM"))

    # ---- constants ----
    wq_sb = const.tile([C, C], d2, tag="wq")
    wk_sb = const.tile([C, C], d2, tag="wk")
    wv_sb = const.tile([C, C], d2, tag="wv")
    pj_sb = const.tile([C, C], d1, tag="pj")
    pjb_sb = const.tile([C, 1], f32, tag="pjb")
    nw_sb = const.tile([C, 1], f32, tag="nw")
    nb_sb = const.tile([C, 1], f32, tag="nb")
    g1_sb = const.tile([C, GROUPS], f32, tag="g1")
    g2_sb = const.tile([GROUPS, C], f32, tag="g2")
    ebc_sb = const.tile([NH, C], d1, tag="ebc")
    for dst, src in ((wq_sb, wq_d), (wk_sb, wk_d), (wv_sb, wv_d),
                     (pj_sb, pjT_d), (pjb_sb, pjb_d), (nw_sb, nw_d),
                     (nb_sb, nb_d), (g1_sb, g1_d), (g2_sb, g2_d),
                     (ebc_sb, ebc_d)):
        nc.sync.dma_start(out=dst, in_=src[:])
    ones_sb = const.tile([1, HW], f32, tag="ones")
    nc.vector.memset(ones_sb, 1.0)
    eps_sb = const.tile([GROUPS, 1], f32, tag="eps")
    nc.vector.memset(eps_sb, EPS)
    ones32 = const.tile([C, DH], f32, tag="ones32")
    nc.vector.memset(ones32, 1.0)


    NHALF = HW // 2  # 512

    for b in range(BPC):
        # ---------- load ----------
        x_sb = sb_x.tile([C, HW], f32, tag="x")
        nc.sync.dma_start(out=x_sb, in_=x_d[b])
        xb2 = sb_x.tile([C, HW], f32, tag="xb2")
        nc.vector.tensor_scalar(out=xb2, in0=x_sb, scalar1=pjb_sb,
                                scalar2=None, op0=OP.add)

        # ---------- groupnorm ----------
        st6 = sb_sm.tile([C, 2, 6], f32, tag="st6")
        nc.vector.bn_stats(out=st6[:, 0, :], in_=x_sb[:, 0:512])
        nc.vector.bn_stats(out=st6[:, 1, :], in_=x_sb[:, 512:1024])
        mv = sb_sm.tile([C, 2], f32, tag="mv")
        nc.vector.bn_aggr(out=mv, in_=st6)
        # s2 = [mean_c, mean_c^2 + var_c]
        s2 = sb_sm.tile([C, 2], f32, tag="s2")
        nc.vector.tensor_copy(out=s2[:, 0:1], in_=mv[:, 0:1])
        nc.vector.tensor_mul(out=s2[:, 1:2], in0=mv[:, 0:1], in1=mv[:, 0:1])
        nc.vector.tensor_add(out=s2[:, 1:2], in0=s2[:, 1:2], in1=mv[:, 1:2])
        # group reduce: [32, 2] = (g1/4)^T @ s2
        gp = ps_s.tile([C, HW], f32, tag="s")
        nc.tensor.matmul(gp[0:GROUPS, 0:2], g1_sb, s2, start=True, stop=True)
        gs = sb_sm.tile([GROUPS, 2], f32, tag="gs")
        nc.vector.tensor_copy(out=gs, in_=gp[0:GROUPS, 0:2])
        # vv = [mu_g, rstd_g]; rstd = exp(-0.5*ln(var+eps)) (same ACT set)
        vv = sb_sm.tile([GROUPS, 2], f32, tag="vv")
        nc.vector.tensor_mul(out=vv[:, 0:1], in0=gs[:, 0:1], in1=gs[:, 0:1])
        nc.vector.tensor_tensor(out=vv[:, 1:2], in0=gs[:, 1:2], in1=vv[:, 0:1],
                                op=OP.subtract)
        nc.scalar.activation(out=vv[:, 1:2], in_=vv[:, 1:2], func=AF.Ln,
                             bias=eps_sb, scale=1.0)
        nc.scalar.activation(out=vv[:, 1:2], in_=vv[:, 1:2], func=AF.Exp,
                             bias=0.0, scale=-0.5)
        nc.vector.tensor_copy(out=vv[:, 0:1], in_=gs[:, 0:1])
        # broadcast to channels: bc[c, 0:2] = [mu_c, rstd_c]
        bc = ps_s.tile([C, HW], f32, tag="s")
        nc.tensor.matmul(bc[0:C, 0:2], g2_sb, vv, start=True, stop=True)
        aff = sb_sm.tile([C, 2], f32, tag="aff")
        nc.vector.tensor_mul(out=aff[:, 0:1], in0=nw_sb, in1=bc[:, 1:2])
        nc.vector.tensor_mul(out=aff[:, 1:2], in0=bc[:, 0:1], in1=aff[:, 0:1])
        nc.vector.tensor_tensor(out=aff[:, 1:2], in0=nb_sb, in1=aff[:, 1:2],
                                op=OP.subtract)
        xn = sb_x.tile([C, HW], d2, tag="xn")
        nc.vector.tensor_scalar(out=xn, in0=x_sb,
                                scalar1=aff[:, 0:1], scalar2=aff[:, 1:2],
                                op0=OP.mult, op1=OP.add)

        # ---------- Q, K, Vt ----------
        qp = ps_s.tile([C, HW], f32, tag="s")
        kp = ps_s.tile([C, HW], f32, tag="s")
        for n in range(2):
            sl = slice(n * NHALF, (n + 1) * NHALF)
            nc.tensor.matmul(qp[:, sl], wq_sb, xn[:, sl], start=True, stop=True)
            nc.tensor.matmul(kp[:, sl], wk_sb, xn[:, sl], start=True, stop=True)
        q_sb = sb_qk.tile([C, HW], d1, tag="q")
        k_sb = sb_qk.tile([C, HW], d1, tag="k")
        nc.vector.tensor_copy(out=q_sb, in_=qp)
        nc.vector.tensor_copy(out=k_sb, in_=kp)

        vp = ps_s.tile([C, HW], f32, tag="s")
        for j in range(8):
            nc.tensor.matmul(vp[:, j * 128:(j + 1) * 128],
                             xn[:, j * 128:(j + 1) * 128], wv_sb,
                             start=True, stop=True)
        vt = sb_qk.tile([C, 8, NH, 33], d1, tag="vt")
        nc.vector.tensor_copy(
            out=vt[:, :, :, 32:33],
            in_=ones32.rearrange("p (a b c) -> p a b c", a=8, b=NH))
        for j in range(8):
            src_v = vp[:, j * 128:(j + 1) * 128].rearrange(
                "p (h d) -> p h d", d=DH)
            nc.vector.tensor_copy(out=vt[:, j, :, 0:DH], in_=src_v)

        # ---------- attention (head pairs; SV = M=33 at base 0) ----------
        hun = sb_x.tile([C, HW], f32, tag="hun")
        zp = sb_sm.tile([C, NH, 8], f32, tag="zp")
        for pair in range(2):
            ha0 = ps_h.tile([33, HW], f32, tag="ha0")
            ha1 = ps_h.tile([33, HW], f32, tag="ha1")
            ha = {0: ha0, 1: ha1}
            for j in range(8):
                a_sb = sb_a.tile([C, 2, HW], d1, tag="a")
                sp0 = ps_s.tile([C, HW], f32, tag="s")
                sp1 = ps_s.tile([C, HW], f32, tag="s")
                sps = (sp0, sp1)
                # S' for both heads first, head-interleaved so the two row
                # groups stream concurrently in the PE array
                for n in range(2):
                    sl = slice(n * NHALF, (n + 1) * NHALF)
                    for k in range(2):
                        h = 2 * pair + k
                        hp = slice(32 * h, 32 * h + 32)
                        nc.tensor.matmul(
                            sps[k][:, sl],
                            k_sb[hp, j * 128:(j + 1) * 128],
                            q_sb[hp, sl],
                            start=True, stop=True,
                            tile_position=(32 * h, 0))
                for k in range(2):
                    nc.scalar.activation(out=a_sb[:, k, :], in_=sps[k],
                                         func=AF.Exp, bias=0.0, scale=1.0)
                for k in range(2):
                    h = 2 * pair + k
                    for n in range(2):
                        sl = slice(n * NHALF, (n + 1) * NHALF)
                        nc.tensor.matmul(
                            ha[k][:, sl],
                            vt[:, j, h, :],
                            a_sb[:, k, sl],
                            start=(j == 0), stop=(j == 7))
            for k in range(2):
                h = 2 * pair + k
                hv = sb_sm.tile([33, HW], f32, tag="hv%d" % k)
                nc.vector.tensor_copy(out=hv, in_=ha[k])
                nc.sync.dma_start(out=zp[:, h, :], in_=hv[32:33, :])
                nc.sync.dma_start(out=hun[32 * h:32 * h + 32, :],
                                  in_=hv[0:32, :])

        # ----- 1/Z and normalize -----
        rp = sb_sm.tile([C, NH, 8], d1, tag="rp")
        with nc.allow_low_precision(reason="1/Z broadcast in bf16; gate 2e-2"):
            nc.vector.reciprocal(out=rp, in_=zp)
        ral = sb_sm.tile([NH, HW], d1, tag="ral")
        for h in range(NH):
            nc.sync.dma_start(out=ral[h:h + 1, :], in_=rp[:, h, :])
        rb_ps = ps_h.tile([C, HW], f32, tag="ha0")
        for n in range(2):
            sl = slice(n * NHALF, (n + 1) * NHALF)
            nc.tensor.matmul(rb_ps[:, sl], ebc_sb, ral[:, sl],
                             start=True, stop=True)
        hn = sb_x.tile([C, HW], d1, tag="hn")
        nc.vector.tensor_mul(out=hn, in0=hun, in1=rb_ps)

        # ---------- proj + bias + residual ----------
        pp = ps_h.tile([C, HW], f32, tag="ha1")
        for n in range(2):
            sl = slice(n * NHALF, (n + 1) * NHALF)
            nc.tensor.matmul(pp[:, sl], pj_sb, hn[:, sl],
                             start=True, stop=True)
        out_sb = sb_x.tile([C, HW], f32, tag="out")
        nc.vector.tensor_add(out=out_sb, in0=pp, in1=xb2)
        nc.sync.dma_start(out=y_d[b], in_=out_sb)


def _get_nc():
    if "nc" not in _CACHE:
        _CACHE["nc"] = _build_nc()
    return _CACHE["nc"]


def _bf16_round(a):
    b = np.ascontiguousarray(a, np.float32).view(np.uint32)
    b = (b + 0x8000 - ((b >> 16) & 1)) & 0xFFFF0000
    return b.view(np.float32)


def _host_prep(inputs):
    x = np.ascontiguousarray(
        np.asarray(inputs["x"], np.float32).reshape(B, C, HW))
    qkv_w = np.asarray(inputs["qkv_w"], np.float32)
    proj_w = np.asarray(inputs["proj_w"], np.float32)
    proj_b = np.asarray(inputs["proj_b"], np.float32)
    norm_w = np.asarray(inputs["norm_w"], np.float32)
    norm_b = np.asarray(inputs["norm_b"], np.float32)

    w3 = qkv_w.reshape(NH, 3, DH, C)  # rows: h*96 + which*32 + d
    wq = w3[:, 0].reshape(C, C)
    wk = w3[:, 1].reshape(C, C)
    wv = w3[:, 2].reshape(C, C)
    wqT = np.ascontiguousarray((wq / 32.0).T)  # fold scale^2 = 1/dh
    wkT = np.ascontiguousarray(wk.T)
    wvT = np.ascontiguousarray(wv.T)
    pjT = np.ascontiguousarray(proj_w.T)
    if PREC == "f32r":
        wqT, wkT, wvT = _bf16_round(wqT), _bf16_round(wkT), _bf16_round(wvT)
        pjT = _bf16_round(pjT)
    elif PREC == "bf16":
        import ml_dtypes
        bf = ml_dtypes.bfloat16
        wqT, wkT, wvT = wqT.astype(bf), wkT.astype(bf), wvT.astype(bf)
        pjT = pjT.astype(bf)

    g1 = np.zeros((C, GROUPS), np.float32)
    g1[np.arange(C), np.arange(C) // 4] = 0.25
    g2 = np.zeros((GROUPS, C), np.float32)
    g2[np.arange(C) // 4, np.arange(C)] = 1.0
    ebc = np.zeros((NH, C), np.float32)
    for h in range(NH):
        ebc[h, 32 * h:32 * h + 32] = 1.0
    if PREC == "bf16":
        import ml_dtypes
        ebc = ebc.astype(ml_dtypes.bfloat16)

    params = dict(
        wqT=wqT, wkT=wkT, wvT=wvT, pjT=pjT,
        pjb=np.ascontiguousarray(proj_b[:, None]),
        nw=np.ascontiguousarray(norm_w[:, None]),
        nb=np.ascontiguousarray(norm_b[:, None]),
        g1=g1, g2=g2, ebc=ebc,
    )
    in_maps = []
    for i in range(NCORES):
        m = dict(params)
        m["x"] = np.ascontiguousarray(x[i * BPC:(i + 1) * BPC])
        in_maps.append(m)
    return in_maps


def kernel(**inputs):
    global LAST_RESULT
    from concourse.bass_utils import run_bass_kernel_spmd
    in_maps = _host_prep(inputs)
    nc = _get_nc()
    res = run_bass_kernel_spmd(nc, in_maps, list(range(NCORES)), trace=TRACE)
    LAST_RESULT = res
    y = np.concatenate([res.results[i]["y"] for i in range(NCORES)], axis=0)
    return y.reshape(B, C, 32, 32)
